# revision 58
# baseline (speedup 1.0000x reference)
"""Multi-head attention (B=2, N=2048, C=1024, H=16) on 8 Trainium2 NeuronCores.

Sharding: tensor-parallel over heads (2 heads/core) for qkv-proj + attention;
all-to-all of the attention output (4 half-batch collectives, pipelined under
attention), then each core runs the output projection over the full channel
dim for its token slices.  Host concatenates slices.

v2 changes vs the 317us baseline:
  - x is pre-transposed on the HOST (numpy) so x^T loads are plain 2D DMAs
    spread across 4 engine DMA queues -- kills the serialized DMA-xbar
    transpose chain that idled the PE for the first 30us.
  - softmax reciprocal via reciprocal_approx_fast (custom DVE op, ~5x faster
    than the iterative divide; 18 bits is plenty for bf16 output).
  - tail: the last half-batch is ONE all-to-all whose input is staged in two
    parts (dst-cores 0-3 right after norm(1,2), rest after norm(1,3)); all
    four output projections run at the end, the first three hidden under the
    final collective.

Per-core structure (heads A=2c, B=2c+1):
  x^T  [c_in, tok]       host-pretransposed, plain DMA
  q/k  [128, tok]        rows 0-63 head A dims, 64-127 head B dims (packed)
  S^T  [128, 1024]       per j-chunk: head A cols 0:512, head B 512:1024
  exp  one ACTIVATE per j-chunk over [128, 1024] PSUM (both heads);
                         S PSUM tiles ping-pong (bufs=2)
  out_u^T [65, i] = [v|1].T @ expS  (row 64 = softmax denominator)
  normalize: reciprocal_approx_fast + DMA broadcast + DVE multiply
"""

import numpy as np
import ml_dtypes
from collections import deque
from contextlib import ExitStack

import concourse.bass as bass
import concourse.tile as tile
from concourse import bacc, mybir
from concourse import hw_specs as _hw_specs
from concourse.bass_utils import run_bass_kernel_spmd
from concourse.masks import make_identity

# The act-table-load pass picks the FIRST table set containing each
# activation function, which puts Exp in `exp_and_others` and Ln in
# `natural_log` and thrashes ~2.7us table loads between them.  Both live
# together in `natural_log_exp_and_others`; steer the pass there by hiding
# Exp/Ln from every other set.  Entry ORDER (= act_func_set_id indexing)
# is preserved, so walrus still resolves the chosen set correctly.
_orig_gat = getattr(_hw_specs, "_bass_kernel_orig_gat", None) \
    or _hw_specs.get_activation_tables
_hw_specs._bass_kernel_orig_gat = _orig_gat


def _patched_gat(module_arch):
    tabs = _orig_gat(module_arch)
    T = mybir.ActivationFunctionType
    for name, fns in tabs.items():
        if name != "natural_log_exp_and_others":
            fns.discard(T.Exp)
            fns.discard(T.Ln)
    return tabs


_hw_specs.get_activation_tables = _patched_gat
bacc.get_activation_tables = _patched_gat

BF16 = mybir.dt.bfloat16
F32 = mybir.dt.float32
EXP = mybir.ActivationFunctionType.Exp
LN = mybir.ActivationFunctionType.Ln
NPBF16 = ml_dtypes.bfloat16

NCORES = 8
B, NSEQ, C, H, D = 2, 2048, 1024, 16, 64
T = B * NSEQ                 # 4096 flattened tokens
SCALE = D ** -0.5            # folded into the exp activation
NKC = C // 128               # 8 contraction chunks
ITILE = 512                  # query tile (free dim of S^T)
NI = NSEQ // ITILE           # 4 i-tiles per batch
NJ = NSEQ // 128             # 16 key chunks per batch
HALF = 1024                  # tokens per all-to-all (half batch)
TFRAG = HALF // NCORES       # 128 tokens per core per all-to-all
TSL = B * NSEQ // NCORES     # 512 output tokens per core

import os
DEBUG_DUMP = os.environ.get("KDBG", "") == "1"


def build_program():
    nc = bacc.Bacc("TRN2", target_bir_lowering=False, debug=False,
                   num_devices=NCORES)

    # x arrives pre-transposed from the host: [C, T]
    xT_d = nc.dram_tensor("x", [C, T], BF16, kind="ExternalInput")
    wqk_d = nc.dram_tensor("wqk", [C, 256], BF16, kind="ExternalInput")
    wv_d = nc.dram_tensor("wv", [C, 128], BF16, kind="ExternalInput")
    wp_d = nc.dram_tensor("wproj", [C, C], BF16, kind="ExternalInput")
    bp_d = nc.dram_tensor("bproj", [1, C], BF16, kind="ExternalInput")
    y_d = nc.dram_tensor("y", [TSL, C], F32, kind="ExternalOutput")

    # batch 0 exchanged as two half-batch collectives (deep-hidden); batch 1
    # as four i-tile-sized ones so the last exposed collective is only 128KB
    a2a_in = [nc.dram_tensor(f"a2a_in{q}", [NCORES * 128, TFRAG], BF16)
              for q in range(2)]
    a2a_out = [nc.dram_tensor(f"a2a_out{q}", [NCORES * 128, TFRAG], BF16)
               for q in range(2)]
    a2a_in_q = [nc.dram_tensor(f"a2a_inq{q}", [NCORES * 128, TFRAG // 2],
                               BF16) for q in range(4)]
    a2a_out_q = [nc.dram_tensor(f"a2a_outq{q}", [NCORES * 128, TFRAG // 2],
                                BF16) for q in range(4)]
    warm_in = nc.dram_tensor("warm_in", [NCORES, 4], BF16)
    warm_out = nc.dram_tensor("warm_out", [NCORES, 4], BF16)
    rcp_d = [nc.dram_tensor(f"rcp_d{s}", [1, ITILE], BF16) for s in range(4)]

    if DEBUG_DUMP:
        dbg_qk = nc.dram_tensor("dbg_qk", [128, 2 * NSEQ], BF16,
                                kind="ExternalOutput")
        dbg_vn = nc.dram_tensor("dbg_vn", [128, NJ * 130], BF16,
                                kind="ExternalOutput")
        dbg_ouc = nc.dram_tensor("dbg_ouc", [65, 8 * ITILE], F32,
                                 kind="ExternalOutput")
        dbg_outT = nc.dram_tensor("dbg_outT", [128, T], BF16,
                                  kind="ExternalOutput")

    with tile.TileContext(nc) as tc, ExitStack() as ctx:
        ep = ctx.enter_context

        consts = ep(tc.tile_pool(name="consts", bufs=1))
        p_exp = ep(tc.tile_pool(name="exps", bufs=4))
        p_ouc = ep(tc.tile_pool(name="ouc", bufs=6))
        p_small = ep(tc.tile_pool(name="small", bufs=4))
        p_ots = ep(tc.tile_pool(name="ots", bufs=4))
        p_y = ep(tc.tile_pool(name="ysb", bufs=2))
        ps_s = ep(tc.tile_pool(name="pss", bufs=2, space="PSUM"))
        ps_ou = ep(tc.tile_pool(name="psou", bufs=2, space="PSUM"))
        ps_mm = ep(tc.tile_pool(name="psmm", bufs=2, space="PSUM"))

        # ---- weights / constants to SBUF ----
        wqk_sb = consts.tile([128, NKC * 256], BF16, name="wqk_sb")
        wv_sb = consts.tile([128, NKC * 128], BF16, name="wv_sb")
        wp_sb = consts.tile([128, NKC * C], BF16, name="wp_sb")
        bp_sb = consts.tile([1, C], BF16, name="bp_sb")

        # x^T: one tile per batch, layout [:, c*2048 + t]
        xt = [consts.tile([128, NKC * NSEQ], BF16, name=f"xt{b}")
              for b in range(B)]

        def load_xu(b, u, cs, eng):
            """Plain 2D DMA of one u-slice (512 tokens) of x^T chunks."""
            for c in cs:
                eng.dma_start(
                    out=xt[b][:, c * NSEQ + u * ITILE:
                              c * NSEQ + (u + 1) * ITILE],
                    in_=xT_d[c * 128:(c + 1) * 128,
                             b * NSEQ + u * ITILE: b * NSEQ + (u + 1) * ITILE])

        def load_wqk(c0, c1, eng):
            eng.dma_start(
                out=wqk_sb[:].rearrange("p (c n) -> p c n",
                                        c=NKC)[:, c0:c1],
                in_=wqk_d[c0 * 128:c1 * 128, :].rearrange(
                    "(c p) n -> p c n", p=128))

        # Startup DMA plan: u-major x slices so each qkv unit's 8 c-chunks
        # arrive together, spread across the three DMA-capable queues
        # (sync/SP, scalar, gpsimd) in first-use order.
        load_wqk(0, 4, nc.sync)
        load_wqk(4, 8, nc.gpsimd)
        # Dummy collective: absorbs the ~11.5us first-collective warmup
        # delay during the qkv phase so a2a(0) starts promptly.
        nc.gpsimd.collective_compute(
            "AllToAll", mybir.AluOpType.bypass,
            replica_groups=[list(range(NCORES))],
            ins=[warm_in[:, :]], outs=[warm_out[:, :]])
        nc.gpsimd.dma_start(
            out=wv_sb[:].rearrange("p (c n) -> p c n", c=NKC),
            in_=wv_d[:, :].rearrange("(c p) n -> p c n", p=128))
        for u in range(4):
            load_xu(0, u, range(0, 4), nc.sync)
            load_xu(0, u, range(4, 8), nc.scalar)
        nc.sync.dma_start(out=bp_sb[:], in_=bp_d[0:1, :])
        for u in range(4):
            load_xu(1, u, range(0, 8), nc.gpsimd)
        nc.sync.dma_start(
            out=wp_sb[:].rearrange("p (c n) -> p c n", c=NKC),
            in_=wp_d[:, :].rearrange("(c p) n -> p c n", p=128))

        ident = consts.tile([128, 128], BF16, name="ident")
        make_identity(nc, ident[:])
        onesc = consts.tile([1, 128], BF16, name="onesc")
        nc.vector.memset(onesc[:], 1.0)

        # bias broadcast [128, C] f32, computed once via 1x128 outer product
        bias_bc = consts.tile([128, C], F32, name="bias_bc")

        def make_bias_bc():
            for n in range(2):
                bps = ps_mm.tile([128, ITILE], F32, tag="mm", name="bps")
                nc.tensor.matmul(bps[:], onesc[:],
                                 bp_sb[:, n * ITILE:(n + 1) * ITILE],
                                 start=True, stop=True)
                nc.vector.tensor_copy(bias_bc[:, n * ITILE:(n + 1) * ITILE],
                                      bps[:])

        # ---- persistent per-batch / per-chunk state ----
        qz = [[consts.tile([128, NSEQ], BF16, name=f"qz{b}{h}")
               for h in range(2)] for b in range(B)]
        kz = [[consts.tile([128, NSEQ], BF16, name=f"kz{b}{h}")
               for h in range(2)] for b in range(B)]
        for b in range(B):
            nc.vector.memset(qz[b][0][64:128, :], 0.0)
            nc.vector.memset(kz[b][0][64:128, :], 0.0)
            nc.vector.memset(qz[b][1][0:64, :], 0.0)
            nc.vector.memset(kz[b][1][0:64, :], 0.0)
        vT = [consts.tile([128, NSEQ], BF16, name=f"vT{b}") for b in range(B)]
        # vn[j]: [v_A(64) | 1 | v_B(64) | 1 | zeros(63)]; constants written once
        vn = [consts.tile([128, 193], BF16, name=f"vn{j}") for j in range(NJ)]
        for j in range(NJ):
            nc.vector.memset(vn[j][:, 64:65], 1.0)
            nc.vector.memset(vn[j][:, 129:130], 1.0)
            nc.vector.memset(vn[j][:, 130:193], 0.0)
        # normalized attention output, per head (partitions 0-63)
        outT = [consts.tile([64, T], BF16, name=f"outT{h}") for h in range(2)]

        def xts(b, u, c):
            return xt[b][:, c * NSEQ + u * ITILE: c * NSEQ + (u + 1) * ITILE]

        # ---- qkv projection: one (w, u) unit = 8 matmuls + 1 evac ----
        def qkv_unit(b, tp, w, uu):
            def emit():
                u = 2 * tp + uu
                usl = slice(u * ITILE, (u + 1) * ITILE)
                pst = ps_mm.tile([128, ITILE], F32, tag="mm", name="pst")
                for c in range(NKC):
                    if w < 2:
                        lhsT = wqk_sb[:, c * 256 + w * 128:
                                      c * 256 + (w + 1) * 128]
                    else:
                        lhsT = wv_sb[:, c * 128:(c + 1) * 128]
                    nc.tensor.matmul(pst[:], lhsT, xts(b, u, c),
                                     start=(c == 0), stop=(c == NKC - 1))
                if w == 2:
                    nc.vector.tensor_copy(vT[b][:, usl], pst[:])
                else:
                    dst = (qz, kz)[w][b]
                    nc.vector.tensor_copy(dst[0][0:64, usl], pst[0:64, :])
                    nc.vector.tensor_copy(dst[1][64:128, usl],
                                          pst[64:128, :])
            return emit

        def qkv_units(b, tp, ws=(0, 1, 2), uu_major=False):
            if uu_major:
                return [qkv_unit(b, tp, w, uu) for uu in range(2) for w in ws]
            return [qkv_unit(b, tp, w, uu) for w in ws for uu in range(2)]

        # ---- vn construction: one unit = 2 transposes + 4 copies ----
        # (PE transposes: DMA-xbar transposes mid-schedule corrupt results,
        # Tile's transpose/collective serialization cannot handle them)
        def vn_unit(b, tcn0):
            def emit():
                for tcn in (tcn0, tcn0 + 1):
                    vtr = ps_mm.tile([128, 128], BF16, tag="mm", name="vtr")
                    nc.tensor.transpose(vtr[:],
                                        vT[b][:, tcn * 128:(tcn + 1) * 128],
                                        ident[:])
                    nc.vector.tensor_copy(vn[tcn][:, 0:64], vtr[:, 0:64])
                    nc.vector.tensor_copy(vn[tcn][:, 65:129], vtr[:, 64:128])
            return emit

        def vn_units(b, tcns):
            return [vn_unit(b, t0) for t0 in tcns]

        # ---- attention ----
        outUc = {}

        def attn_pairs(b, i, sub=None):
            """Generator: one yield per j-chunk pair (8 per i-tile).
            sub=0/1 processes only a 256-query half of the i-tile (used to
            shrink the final exchanged fragment)."""
            if sub is None:
                q0, W = i * ITILE, ITILE
            else:
                q0, W = i * ITILE + sub * (ITILE // 2), ITILE // 2
            isl = slice(q0, q0 + W)
            outu = [ps_ou.tile([128, W], F32, tag="ou", name="outu")
                    for _ in range(2)]
            for g in range(NJ // 2):
                sts = []
                for jj in (2 * g, 2 * g + 1):
                    s_t = ps_s.tile([128, 2 * W], F32, tag="s", name="s_t")
                    for h in range(2):
                        nc.tensor.matmul(
                            s_t[:, h * W:(h + 1) * W],
                            kz[b][h][:, jj * 128:(jj + 1) * 128],
                            qz[b][h][:, isl],
                            start=True, stop=True)
                    sts.append(s_t)
                exs = []
                for k in range(2):
                    ex = p_exp.tile([128, 2 * W], BF16, tag="ex", name="ex")
                    nc.scalar.activation(ex[:], sts[k][:], EXP, scale=SCALE)
                    exs.append(ex)
                for k, jj in enumerate((2 * g, 2 * g + 1)):
                    for h in range(2):
                        nc.tensor.matmul(
                            outu[h][:],
                            vn[jj][:, h * 65: h * 65 + 128],
                            exs[k][:, h * W:(h + 1) * W],
                            start=(jj == 0), stop=(jj == NJ - 1))
                yield
            last = (b, i) == (B - 1, NI - 1) and sub in (None, 1)
            for h in range(2):
                # 1/den = exp(-ln(den)) on ScalarE straight from the PSUM
                # denominator row (Ln and Exp share one activation table
                # set, natural_log_exp_and_others -- no table switching).
                lnt = p_small.tile([65, W], F32, tag="rcp", name="lnt")
                nc.scalar.activation(lnt[64:65, :], outu[h][64:65, :], LN)
                rcpb = p_small.tile([65, W], BF16, tag="rcpb",
                                    name="rcpb")
                nc.scalar.activation(rcpb[64:65, :], lnt[64:65, :], EXP,
                                     scale=-1.0)
                # write 1/den to DRAM now; the partition broadcast happens
                # in norm_mul (DMA cannot broadcast from an SBUF source)
                slot = (b * 8 + i * 2 + h) % 4
                nc.sync.dma_start(out=rcp_d[slot][0:1, 0:W],
                                  in_=rcpb[64:65, :])
                if last and not DEBUG_DUMP:
                    # last sub-tile: nothing reuses this PSUM afterwards --
                    # skip the evac copy and let norm_mul read PSUM
                    # directly (shaves the copy off the tail chain)
                    outUc[(b, i, h, sub)] = (outu[h], slot, q0, W)
                    continue
                ouc = p_ouc.tile([65, W], F32, tag="ouc", name="ouc")
                nc.vector.tensor_copy(ouc[:], outu[h][0:65, :])
                outUc[(b, i, h, sub)] = (ouc, slot, q0, W)
                if DEBUG_DUMP and b == 0:
                    sl = (i * 2 + h) * ITILE
                    nc.sync.dma_start(out=dbg_ouc[:, sl:sl + W],
                                      in_=ouc[:])

        def norm_mul(b, i, h, sub=None):
            """DMA broadcast of 1/den + DVE multiply; scheduled a few slots
            after the i-tile so the rcp_d write latency is hidden."""
            def emit():
                t0 = b * NSEQ
                ouc, slot, q0, W = outUc.pop((b, i, h, sub))
                bc_sb = p_small.tile([64, W], BF16, tag="bc", name="bcsb")
                nc.sync.dma_start(
                    out=bc_sb[:],
                    in_=rcp_d[slot][0:1, 0:W].to_broadcast((64, W)))
                nc.vector.tensor_mul(
                    outT[h][0:64, t0 + q0: t0 + q0 + W],
                    ouc[0:64, :], bc_sb[:])
            return emit

        # All CC-gated DMAs (ots loads, y stores) live on the gpsimd DGE
        # queue in natural order (trigger q -> ots q -> trigger q+1 ...):
        # their semaphore waits are harmless there because everything queued
        # behind them is gated even later.  On sync they head-of-line-block
        # the norm broadcasts; on scalar they stall the exp pipeline.
        otsT = {}

        def stage_a2a(q):
            """Half-batch collective for batch 0 (tokens q*1024..)."""
            hs = q * HALF
            for h in range(2):
                nc.sync.dma_start(
                    out=a2a_in[q][:, :].rearrange(
                        "(s x) t -> x s t", s=NCORES)[h * 64:(h + 1) * 64],
                    in_=outT[h][:, hs: hs + HALF].rearrange(
                        "p (s t) -> p s t", s=NCORES))
            nc.gpsimd.collective_compute(
                "AllToAll", mybir.AluOpType.bypass,
                replica_groups=[list(range(NCORES))],
                ins=[a2a_in[q][:, :]], outs=[a2a_out[q][:, :]])
            ots = p_ots.tile([128, NCORES * TFRAG], BF16, tag="ots",
                             name="ots")
            nc.gpsimd.dma_start(
                out=ots[:].rearrange("p (s t) -> p s t", s=NCORES),
                in_=a2a_out[q][:, :].rearrange("(s p) t -> p s t",
                                               s=NCORES))
            otsT[q] = ots

        def stage_a2a_q(qq):
            """i-tile-sized collective for batch 1 (tokens 2048+qq*512..)."""
            hs = NSEQ + qq * ITILE
            for h in range(2):
                nc.sync.dma_start(
                    out=a2a_in_q[qq][:, :].rearrange(
                        "(s x) t -> x s t", s=NCORES)[h * 64:(h + 1) * 64],
                    in_=outT[h][:, hs: hs + ITILE].rearrange(
                        "p (s t) -> p s t", s=NCORES))
            nc.gpsimd.collective_compute(
                "AllToAll", mybir.AluOpType.bypass,
                replica_groups=[list(range(NCORES))],
                ins=[a2a_in_q[qq][:, :]], outs=[a2a_out_q[qq][:, :]])

        def load_ots_q2(qq0):
            """Pair two 64-token fragments (qq0, qq0+1) into one ots tile."""
            ots = p_ots.tile([128, NCORES * TFRAG], BF16, tag="ots",
                             name="otsq")
            tf = TFRAG // 2
            for k in range(2):
                nc.gpsimd.dma_start(
                    out=ots[:].rearrange("p (s t) -> p s t",
                                         s=NCORES)[:, :, k * tf:(k + 1) * tf],
                    in_=a2a_out_q[qq0 + k][:, :].rearrange(
                        "(s p) t -> p s t", s=NCORES))
            otsT[2 + qq0 // 2] = ots

        def oproj(q):
            # alternate the PSUM pool (ps_mm / ps_ou, both idle in the
            # tail) so consecutive oproj units don't convoy on the 2-buffer
            # ring waiting for the previous unit's bias-adds
            def emit():
                ots = otsT.pop(q)
                pool, tg = [(ps_mm, "mm"), (ps_ou, "ou"),
                            (ps_s, "s"), (ps_ou, "ou")][q]
                y_ps = [pool.tile([128, ITILE], F32, tag=tg, name="yps")
                        for _ in range(2)]
                y_sb = p_y.tile([128, C], F32, tag="y", name="ysb")
                yr0 = q * TFRAG
                # n-outer so the first half's bias-add + y store overlap
                # the second half's matmul chain (shrinks the exposed tail)
                for n in range(2):
                    for s in range(NKC):
                        nc.tensor.matmul(
                            y_ps[n][:],
                            ots[:, s * TFRAG:(s + 1) * TFRAG],
                            wp_sb[:, s * C + n * ITILE: s * C + (n + 1) * ITILE],
                            start=(s == 0), stop=(s == NKC - 1))
                    nc.vector.tensor_add(y_sb[:, n * ITILE:(n + 1) * ITILE],
                                         y_ps[n][:],
                                         bias_bc[:, n * ITILE:(n + 1) * ITILE])
                    # y stores go on sync: on gpsimd they sit behind
                    # CC-gated ots loads and keep y_sb ring buffers alive
                    nc.sync.dma_start(
                        out=y_d[yr0: yr0 + TFRAG, n * ITILE:(n + 1) * ITILE],
                        in_=y_sb[:, n * ITILE:(n + 1) * ITILE])
            return emit

        def drive(gen, fillers):
            """Interleave: one filler unit emitted BEFORE each attention
            j-pair; leftovers drain after the i-tile. None = empty slot."""
            fl = deque(fillers)
            while True:
                if fl:
                    f = fl.popleft()
                    if f is not None:
                        f()
                try:
                    next(gen)
                except StopIteration:
                    break
            while fl:
                f = fl.popleft()
                if f is not None:
                    f()

        def seq(*fns):
            def emit():
                for f in fns:
                    f()
            return emit

        def a2a_unit(q):
            def emit():
                stage_a2a(q)
            return emit

        # ---- emission schedule ----
        for f in qkv_units(0, 0, uu_major=True):
            f()
        for f in vn_units(0, range(0, 8, 2)):
            f()

        drive(attn_pairs(0, 0),
              qkv_units(0, 1, ws=(2, 1)) + vn_units(0, range(8, 16, 2))
              + qkv_units(0, 1, ws=(0,)))
        if DEBUG_DUMP:
            nc.sync.dma_start(out=dbg_qk[0:64, 0:NSEQ], in_=qz[0][0][0:64, :])
            nc.sync.dma_start(out=dbg_qk[64:128, 0:NSEQ],
                              in_=qz[0][1][64:128, :])
            nc.sync.dma_start(out=dbg_qk[0:64, NSEQ:], in_=kz[0][0][0:64, :])
            nc.sync.dma_start(out=dbg_qk[64:128, NSEQ:],
                              in_=kz[0][1][64:128, :])
            for j in range(NJ):
                nc.sync.dma_start(out=dbg_vn[:, j * 130:j * 130 + 130],
                                  in_=vn[j][:, 0:130])

        drive(attn_pairs(0, 1),
              [make_bias_bc, None,
               norm_mul(0, 0, 0), norm_mul(0, 0, 1)])
        q10 = qkv_units(1, 0)
        drive(attn_pairs(0, 2),
              [q10[0], q10[1], None,
               seq(norm_mul(0, 1, 0), norm_mul(0, 1, 1), a2a_unit(0)),
               q10[2], q10[3], q10[4], q10[5]])
        # vn_units(1, (0,1)) must sit at slot >= 1: the slot-k filler is
        # emitted BEFORE attention pair k, and pair 0 still reads batch-0
        # vn[0..1] -- emitting the overwrite first would reorder the data.
        drive(attn_pairs(0, 3),
              [norm_mul(0, 2, 0)] + vn_units(1, range(0, 8, 2))
              + [norm_mul(0, 2, 1)])
        qk11 = qkv_units(1, 1, ws=(2, 1))
        vn1hi = vn_units(1, range(8, 16, 2))
        q11q = qkv_units(1, 1, ws=(0,))
        drive(attn_pairs(1, 0),
              [qk11[0], qk11[1], None,
               seq(norm_mul(0, 3, 0), norm_mul(0, 3, 1), a2a_unit(1)),
               seq(qk11[2], vn1hi[0]), seq(qk11[3], vn1hi[1]),
               vn1hi[2], vn1hi[3], q11q[0], q11q[1]])
        drive(attn_pairs(1, 1),
              [None, None,
               seq(norm_mul(1, 0, 0), norm_mul(1, 0, 1)),
               lambda: stage_a2a_q(0)])
        drive(attn_pairs(1, 2),
              [None, None, None,
               seq(norm_mul(1, 1, 0), norm_mul(1, 1, 1)),
               lambda: (stage_a2a_q(1), load_ots_q2(0))])
        drive(attn_pairs(1, 3),
              [None, None, None,
               seq(norm_mul(1, 2, 0), norm_mul(1, 2, 1)),
               lambda: stage_a2a_q(2)])
        norm_mul(1, 3, 0)(); norm_mul(1, 3, 1)()
        if DEBUG_DUMP:
            nc.sync.dma_start(out=dbg_outT[0:64, :], in_=outT[0][:])
            nc.sync.dma_start(out=dbg_outT[64:128, :], in_=outT[1][:])
        stage_a2a_q(3)
        # deferred output projections: 0-2 fill the PE while the final
        # collective is in flight; 3 runs as soon as its data lands.
        # tile_wait_until pins them to the END of the static schedule --
        # otherwise the scheduler interleaves their matmuls into the
        # attention stream (their a2a data is long ready), delaying the
        # last ACTIVATE that gates the final collective by ~13us, and
        # reorders the ots loads ahead of the final collective trigger on
        # the gpsimd queue.  Runtime order is still semaphore-driven.
        with tc.tile_wait_until(1.0):
            load_ots_q2(2)
            oproj(0)()
            oproj(1)()
            oproj(2)()
            oproj(3)()

    nc.compile()
    return nc


_NC = None


def _get_nc():
    global _NC
    if _NC is None:
        _NC = build_program()
    return _NC


def prep_in_maps(x, w_qkv, w_proj, b_proj):
    x = np.asarray(x, dtype=np.float32).reshape(T, C)
    xT_bf = np.ascontiguousarray(x.T).astype(NPBF16)          # [C, T]
    w_qkv = np.asarray(w_qkv, dtype=np.float32)
    w_proj = np.asarray(w_proj, dtype=np.float32)
    b_proj = np.asarray(b_proj, dtype=np.float32)
    wp_bf = np.ascontiguousarray(w_proj).astype(NPBF16)
    bp_bf = b_proj.reshape(1, C).astype(NPBF16)

    q_w, k_w, v_w = w_qkv[:, 0:C], w_qkv[:, C:2 * C], w_qkv[:, 2 * C:3 * C]
    in_maps = []
    for c in range(NCORES):
        hA, hB = 2 * c, 2 * c + 1
        sA, sB = slice(hA * D, (hA + 1) * D), slice(hB * D, (hB + 1) * D)
        wqk_c = np.concatenate([q_w[:, sA], q_w[:, sB], k_w[:, sA], k_w[:, sB]],
                               axis=1).astype(NPBF16)
        wv_c = np.concatenate([v_w[:, sA], v_w[:, sB]], axis=1).astype(NPBF16)
        in_maps.append({"x": xT_bf, "wqk": np.ascontiguousarray(wqk_c),
                        "wv": np.ascontiguousarray(wv_c), "wproj": wp_bf,
                        "bproj": bp_bf})
    return in_maps


def assemble(results):
    y = np.empty((T, C), dtype=np.float32)
    tf = TFRAG // 2
    for c in range(NCORES):
        yc = results[c]["y"]
        # batch 0: two half-batch fragments of 128 tokens
        for q in range(2):
            g0 = q * HALF + c * TFRAG
            y[g0: g0 + TFRAG, :] = yc[q * TFRAG: (q + 1) * TFRAG, :]
        # batch 1: four i-tile fragments of 64 tokens
        for qq in range(4):
            g0 = NSEQ + qq * ITILE + c * tf
            r0 = 2 * TFRAG + qq * tf
            y[g0: g0 + tf, :] = yc[r0: r0 + tf, :]
    return y.reshape(B, NSEQ, C)


def run(in_maps, trace=False):
    nc = _get_nc()
    return run_bass_kernel_spmd(nc, in_maps, core_ids=list(range(NCORES)),
                                trace=trace)


def kernel(x, w_qkv, w_proj, b_proj):
    res = run(prep_in_maps(x, w_qkv, w_proj, b_proj))
    return assemble(res.results)


# revision 60
# speedup vs baseline: 1.0423x; 1.0423x over previous
"""Multi-head attention (B=2, N=2048, C=1024, H=16) on 8 Trainium2 NeuronCores.

Sharding: tensor-parallel over heads (2 heads/core) for qkv-proj + attention;
all-to-all of the attention output (4 half-batch collectives, pipelined under
attention), then each core runs the output projection over the full channel
dim for its token slices.  Host concatenates slices.

v2 changes vs the 317us baseline:
  - x is pre-transposed on the HOST (numpy) so x^T loads are plain 2D DMAs
    spread across 4 engine DMA queues -- kills the serialized DMA-xbar
    transpose chain that idled the PE for the first 30us.
  - softmax reciprocal via reciprocal_approx_fast (custom DVE op, ~5x faster
    than the iterative divide; 18 bits is plenty for bf16 output).
  - tail: the last half-batch is ONE all-to-all whose input is staged in two
    parts (dst-cores 0-3 right after norm(1,2), rest after norm(1,3)); all
    four output projections run at the end, the first three hidden under the
    final collective.

Per-core structure (heads A=2c, B=2c+1):
  x^T  [c_in, tok]       host-pretransposed, plain DMA
  q/k  [128, tok]        rows 0-63 head A dims, 64-127 head B dims (packed)
  S^T  [128, 1024]       per j-chunk: head A cols 0:512, head B 512:1024
  exp  one ACTIVATE per j-chunk over [128, 1024] PSUM (both heads);
                         S PSUM tiles ping-pong (bufs=2)
  out_u^T [65, i] = [v|1].T @ expS  (row 64 = softmax denominator)
  normalize: reciprocal_approx_fast + DMA broadcast + DVE multiply
"""

import numpy as np
import ml_dtypes
from collections import deque
from contextlib import ExitStack

import concourse.bass as bass
import concourse.tile as tile
from concourse import bacc, mybir
from concourse import hw_specs as _hw_specs
from concourse.bass_utils import run_bass_kernel_spmd
from concourse.masks import make_identity

# The act-table-load pass picks the FIRST table set containing each
# activation function, which puts Exp in `exp_and_others` and Ln in
# `natural_log` and thrashes ~2.7us table loads between them.  Both live
# together in `natural_log_exp_and_others`; steer the pass there by hiding
# Exp/Ln from every other set.  Entry ORDER (= act_func_set_id indexing)
# is preserved, so walrus still resolves the chosen set correctly.
_orig_gat = getattr(_hw_specs, "_bass_kernel_orig_gat", None) \
    or _hw_specs.get_activation_tables
_hw_specs._bass_kernel_orig_gat = _orig_gat


def _patched_gat(module_arch):
    tabs = _orig_gat(module_arch)
    T = mybir.ActivationFunctionType
    for name, fns in tabs.items():
        if name != "natural_log_exp_and_others":
            fns.discard(T.Exp)
            fns.discard(T.Ln)
    return tabs


_hw_specs.get_activation_tables = _patched_gat
bacc.get_activation_tables = _patched_gat

BF16 = mybir.dt.bfloat16
F32 = mybir.dt.float32
EXP = mybir.ActivationFunctionType.Exp
LN = mybir.ActivationFunctionType.Ln
NPBF16 = ml_dtypes.bfloat16

NCORES = 8
B, NSEQ, C, H, D = 2, 2048, 1024, 16, 64
T = B * NSEQ                 # 4096 flattened tokens
SCALE = D ** -0.5            # folded into the exp activation
NKC = C // 128               # 8 contraction chunks
ITILE = 512                  # query tile (free dim of S^T)
NI = NSEQ // ITILE           # 4 i-tiles per batch
NJ = NSEQ // 128             # 16 key chunks per batch
HALF = 1024                  # tokens per all-to-all (half batch)
TFRAG = HALF // NCORES       # 128 tokens per core per all-to-all
TSL = B * NSEQ // NCORES     # 512 output tokens per core

import os
DEBUG_DUMP = os.environ.get("KDBG", "") == "1"


def build_program():
    nc = bacc.Bacc("TRN2", target_bir_lowering=False, debug=False,
                   num_devices=NCORES)

    # x arrives pre-transposed from the host: [C, T]
    xT_d = nc.dram_tensor("x", [C, T], BF16, kind="ExternalInput")
    wqk_d = nc.dram_tensor("wqk", [C, 256], BF16, kind="ExternalInput")
    wv_d = nc.dram_tensor("wv", [C, 128], BF16, kind="ExternalInput")
    wp_d = nc.dram_tensor("wproj", [C, C], BF16, kind="ExternalInput")
    bp_d = nc.dram_tensor("bproj", [1, C], BF16, kind="ExternalInput")
    y_d = nc.dram_tensor("y", [TSL, C], F32, kind="ExternalOutput")

    # batch 0 exchanged as two half-batch collectives (deep-hidden); batch 1
    # as four i-tile-sized ones so the last exposed collective is only 128KB
    a2a_in = [nc.dram_tensor(f"a2a_in{q}", [NCORES * 128, TFRAG], BF16)
              for q in range(2)]
    a2a_out = [nc.dram_tensor(f"a2a_out{q}", [NCORES * 128, TFRAG], BF16)
               for q in range(2)]
    a2a_in_q = [nc.dram_tensor(f"a2a_inq{q}", [NCORES * 128, TFRAG // 2],
                               BF16) for q in range(4)]
    a2a_out_q = [nc.dram_tensor(f"a2a_outq{q}", [NCORES * 128, TFRAG // 2],
                                BF16) for q in range(4)]
    warm_in = nc.dram_tensor("warm_in", [NCORES, 4], BF16)
    warm_out = nc.dram_tensor("warm_out", [NCORES, 4], BF16)
    rcp_d = [nc.dram_tensor(f"rcp_d{s}", [1, ITILE], BF16) for s in range(4)]

    if DEBUG_DUMP:
        dbg_qk = nc.dram_tensor("dbg_qk", [128, 2 * NSEQ], BF16,
                                kind="ExternalOutput")
        dbg_vn = nc.dram_tensor("dbg_vn", [128, NJ * 130], BF16,
                                kind="ExternalOutput")
        dbg_ouc = nc.dram_tensor("dbg_ouc", [65, 8 * ITILE], F32,
                                 kind="ExternalOutput")
        dbg_outT = nc.dram_tensor("dbg_outT", [128, T], BF16,
                                  kind="ExternalOutput")

    with tile.TileContext(nc) as tc, ExitStack() as ctx:
        ep = ctx.enter_context

        consts = ep(tc.tile_pool(name="consts", bufs=1))
        p_exp = ep(tc.tile_pool(name="exps", bufs=4))
        p_ouc = ep(tc.tile_pool(name="ouc", bufs=6))
        p_small = ep(tc.tile_pool(name="small", bufs=4))
        p_ots = ep(tc.tile_pool(name="ots", bufs=4))
        p_y = ep(tc.tile_pool(name="ysb", bufs=2))
        ps_s = ep(tc.tile_pool(name="pss", bufs=2, space="PSUM"))
        ps_ou = ep(tc.tile_pool(name="psou", bufs=2, space="PSUM"))
        ps_mm = ep(tc.tile_pool(name="psmm", bufs=2, space="PSUM"))

        # ---- weights / constants to SBUF ----
        wqk_sb = consts.tile([128, NKC * 256], BF16, name="wqk_sb")
        wv_sb = consts.tile([128, NKC * 128], BF16, name="wv_sb")
        wp_sb = consts.tile([128, NKC * C], BF16, name="wp_sb")
        bp_sb = consts.tile([1, C], BF16, name="bp_sb")

        # x^T: one tile per batch, layout [:, c*2048 + t]
        xt = [consts.tile([128, NKC * NSEQ], BF16, name=f"xt{b}")
              for b in range(B)]

        def load_xu(b, u, cs, eng):
            """Plain 2D DMA of one u-slice (512 tokens) of x^T chunks."""
            for c in cs:
                eng.dma_start(
                    out=xt[b][:, c * NSEQ + u * ITILE:
                              c * NSEQ + (u + 1) * ITILE],
                    in_=xT_d[c * 128:(c + 1) * 128,
                             b * NSEQ + u * ITILE: b * NSEQ + (u + 1) * ITILE])

        def load_wqk(c0, c1, eng):
            eng.dma_start(
                out=wqk_sb[:].rearrange("p (c n) -> p c n",
                                        c=NKC)[:, c0:c1],
                in_=wqk_d[c0 * 128:c1 * 128, :].rearrange(
                    "(c p) n -> p c n", p=128))

        # Startup DMA plan: u-major x slices so each qkv unit's 8 c-chunks
        # arrive together, spread across the three DMA-capable queues
        # (sync/SP, scalar, gpsimd) in first-use order.
        load_wqk(0, 4, nc.sync)
        load_wqk(4, 8, nc.gpsimd)
        # Dummy collective: absorbs the ~11.5us first-collective warmup
        # delay during the qkv phase so a2a(0) starts promptly.
        nc.gpsimd.collective_compute(
            "AllToAll", mybir.AluOpType.bypass,
            replica_groups=[list(range(NCORES))],
            ins=[warm_in[:, :]], outs=[warm_out[:, :]])
        nc.gpsimd.dma_start(
            out=wv_sb[:].rearrange("p (c n) -> p c n", c=NKC),
            in_=wv_d[:, :].rearrange("(c p) n -> p c n", p=128))
        for u in range(4):
            load_xu(0, u, range(0, 4), nc.sync)
            load_xu(0, u, range(4, 8), nc.scalar)
        nc.sync.dma_start(out=bp_sb[:], in_=bp_d[0:1, :])
        for u in range(4):
            load_xu(1, u, range(0, 8), nc.gpsimd)
        nc.sync.dma_start(
            out=wp_sb[:].rearrange("p (c n) -> p c n", c=NKC),
            in_=wp_d[:, :].rearrange("(c p) n -> p c n", p=128))

        ident = consts.tile([128, 128], BF16, name="ident")
        make_identity(nc, ident[:])
        onesc = consts.tile([1, 128], BF16, name="onesc")
        nc.vector.memset(onesc[:], 1.0)

        # bias broadcast [128, C] f32, computed once via 1x128 outer product
        bias_bc = consts.tile([128, C], F32, name="bias_bc")

        def make_bias_bc():
            for n in range(2):
                bps = ps_mm.tile([128, ITILE], F32, tag="mm", name="bps")
                nc.tensor.matmul(bps[:], onesc[:],
                                 bp_sb[:, n * ITILE:(n + 1) * ITILE],
                                 start=True, stop=True)
                nc.vector.tensor_copy(bias_bc[:, n * ITILE:(n + 1) * ITILE],
                                      bps[:])

        # ---- persistent per-batch / per-chunk state ----
        qz = [[consts.tile([128, NSEQ], BF16, name=f"qz{b}{h}")
               for h in range(2)] for b in range(B)]
        kz = [[consts.tile([128, NSEQ], BF16, name=f"kz{b}{h}")
               for h in range(2)] for b in range(B)]
        for b in range(B):
            nc.vector.memset(qz[b][0][64:128, :], 0.0)
            nc.vector.memset(kz[b][0][64:128, :], 0.0)
            nc.vector.memset(qz[b][1][0:64, :], 0.0)
            nc.vector.memset(kz[b][1][0:64, :], 0.0)
        vT = [consts.tile([128, NSEQ], BF16, name=f"vT{b}") for b in range(B)]
        # vn[j]: [v_A(64) | 1 | v_B(64) | 1 | zeros(63)]; constants written once
        vn = [consts.tile([128, 193], BF16, name=f"vn{j}") for j in range(NJ)]
        for j in range(NJ):
            nc.vector.memset(vn[j][:, 64:65], 1.0)
            nc.vector.memset(vn[j][:, 129:130], 1.0)
            nc.vector.memset(vn[j][:, 130:193], 0.0)
        # normalized attention output, per head (partitions 0-63)
        outT = [consts.tile([64, T], BF16, name=f"outT{h}") for h in range(2)]

        def xts(b, u, c):
            return xt[b][:, c * NSEQ + u * ITILE: c * NSEQ + (u + 1) * ITILE]

        # ---- qkv projection: one (w, u) unit = 8 matmuls + 1 evac ----
        def qkv_unit(b, tp, w, uu):
            def emit():
                u = 2 * tp + uu
                usl = slice(u * ITILE, (u + 1) * ITILE)
                pst = ps_mm.tile([128, ITILE], F32, tag="mm", name="pst")
                for c in range(NKC):
                    if w < 2:
                        lhsT = wqk_sb[:, c * 256 + w * 128:
                                      c * 256 + (w + 1) * 128]
                    else:
                        lhsT = wv_sb[:, c * 128:(c + 1) * 128]
                    nc.tensor.matmul(pst[:], lhsT, xts(b, u, c),
                                     start=(c == 0), stop=(c == NKC - 1))
                if w == 2:
                    nc.vector.tensor_copy(vT[b][:, usl], pst[:])
                else:
                    dst = (qz, kz)[w][b]
                    nc.vector.tensor_copy(dst[0][0:64, usl], pst[0:64, :])
                    nc.vector.tensor_copy(dst[1][64:128, usl],
                                          pst[64:128, :])
            return emit

        def qkv_units(b, tp, ws=(0, 1, 2), uu_major=False):
            if uu_major:
                return [qkv_unit(b, tp, w, uu) for uu in range(2) for w in ws]
            return [qkv_unit(b, tp, w, uu) for w in ws for uu in range(2)]

        # ---- vn construction: one unit = 2 transposes + 4 copies ----
        # (PE transposes: DMA-xbar transposes mid-schedule corrupt results,
        # Tile's transpose/collective serialization cannot handle them)
        def vn_unit(b, tcn0):
            def emit():
                for tcn in (tcn0, tcn0 + 1):
                    vtr = ps_mm.tile([128, 128], BF16, tag="mm", name="vtr")
                    nc.tensor.transpose(vtr[:],
                                        vT[b][:, tcn * 128:(tcn + 1) * 128],
                                        ident[:])
                    nc.vector.tensor_copy(vn[tcn][:, 0:64], vtr[:, 0:64])
                    nc.vector.tensor_copy(vn[tcn][:, 65:129], vtr[:, 64:128])
            return emit

        def vn_units(b, tcns):
            return [vn_unit(b, t0) for t0 in tcns]

        # ---- attention ----
        outUc = {}

        def attn_pairs(b, i, sub=None):
            """Generator: one yield per j-chunk pair (8 per i-tile).
            sub=0/1 processes only a 256-query half of the i-tile (used to
            shrink the final exchanged fragment)."""
            if sub is None:
                q0, W = i * ITILE, ITILE
            else:
                q0, W = i * ITILE + sub * (ITILE // 2), ITILE // 2
            isl = slice(q0, q0 + W)
            outu = [ps_ou.tile([128, W], F32, tag="ou", name="outu")
                    for _ in range(2)]
            for g in range(NJ // 2):
                sts = []
                for jj in (2 * g, 2 * g + 1):
                    s_t = ps_s.tile([128, 2 * W], F32, tag="s", name="s_t")
                    for h in range(2):
                        nc.tensor.matmul(
                            s_t[:, h * W:(h + 1) * W],
                            kz[b][h][:, jj * 128:(jj + 1) * 128],
                            qz[b][h][:, isl],
                            start=True, stop=True)
                    sts.append(s_t)
                exs = []
                for k in range(2):
                    ex = p_exp.tile([128, 2 * W], BF16, tag="ex", name="ex")
                    nc.scalar.activation(ex[:], sts[k][:], EXP, scale=SCALE)
                    exs.append(ex)
                for k, jj in enumerate((2 * g, 2 * g + 1)):
                    for h in range(2):
                        nc.tensor.matmul(
                            outu[h][:],
                            vn[jj][:, h * 65: h * 65 + 128],
                            exs[k][:, h * W:(h + 1) * W],
                            start=(jj == 0), stop=(jj == NJ - 1))
                yield
            last = (b, i) == (B - 1, NI - 1) and sub in (None, 1)
            for h in range(2):
                slot = (b * 8 + i * 2 + h) % 4
                if last and not DEBUG_DUMP:
                    # last sub-tile: run the whole reciprocal chain inline
                    # (tail latency) and read PSUM directly -- nothing
                    # reuses this PSUM afterwards, no evac copy needed.
                    lnt = p_small.tile([65, W], F32, tag="rcp", name="lnt")
                    nc.scalar.activation(lnt[64:65, :], outu[h][64:65, :],
                                         LN)
                    rcpb = p_small.tile([65, W], BF16, tag="rcpb",
                                        name="rcpb")
                    nc.scalar.activation(rcpb[64:65, :], lnt[64:65, :],
                                         EXP, scale=-1.0)
                    nc.sync.dma_start(out=rcp_d[slot][0:1, 0:W],
                                      in_=rcpb[64:65, :])
                    outUc[(b, i, h, sub)] = (outu[h], slot, q0, W)
                    continue
                ouc = p_ouc.tile([65, W], F32, tag="ouc", name="ouc")
                nc.vector.tensor_copy(ouc[:], outu[h][0:65, :])
                outUc[(b, i, h, sub)] = (ouc, slot, q0, W)
                if DEBUG_DUMP and b == 0:
                    sl = (i * 2 + h) * ITILE
                    nc.sync.dma_start(out=dbg_ouc[:, sl:sl + W],
                                      in_=ouc[:])

        def norm_rcp(b, i, h, sub=None):
            """1/den = exp(-ln(den)) on ScalarE from the ouc copy.  Emitted
            as a filler in the NEXT drive so the two extra ScalarE ops don't
            wedge between two i-tiles' exp streams and stall the AV refill
            (Ln and Exp share one table set -- no table switching)."""
            def emit():
                ouc, slot, q0, W = outUc[(b, i, h, sub)]
                lnt = p_small.tile([65, W], F32, tag="rcp", name="lnt")
                nc.scalar.activation(lnt[64:65, :], ouc[64:65, :], LN)
                rcpb = p_small.tile([65, W], BF16, tag="rcpb", name="rcpb")
                nc.scalar.activation(rcpb[64:65, :], lnt[64:65, :], EXP,
                                     scale=-1.0)
                nc.sync.dma_start(out=rcp_d[slot][0:1, 0:W],
                                  in_=rcpb[64:65, :])
            return emit

        def norm_mul(b, i, h, sub=None):
            """DMA broadcast of 1/den + DVE multiply; scheduled a few slots
            after the i-tile so the rcp_d write latency is hidden."""
            def emit():
                t0 = b * NSEQ
                ouc, slot, q0, W = outUc.pop((b, i, h, sub))
                bc_sb = p_small.tile([64, W], BF16, tag="bc", name="bcsb")
                nc.sync.dma_start(
                    out=bc_sb[:],
                    in_=rcp_d[slot][0:1, 0:W].to_broadcast((64, W)))
                nc.vector.tensor_mul(
                    outT[h][0:64, t0 + q0: t0 + q0 + W],
                    ouc[0:64, :], bc_sb[:])
            return emit

        # All CC-gated DMAs (ots loads, y stores) live on the gpsimd DGE
        # queue in natural order (trigger q -> ots q -> trigger q+1 ...):
        # their semaphore waits are harmless there because everything queued
        # behind them is gated even later.  On sync they head-of-line-block
        # the norm broadcasts; on scalar they stall the exp pipeline.
        otsT = {}

        def stage_a2a(q):
            """Half-batch collective for batch 0 (tokens q*1024..)."""
            hs = q * HALF
            for h in range(2):
                nc.sync.dma_start(
                    out=a2a_in[q][:, :].rearrange(
                        "(s x) t -> x s t", s=NCORES)[h * 64:(h + 1) * 64],
                    in_=outT[h][:, hs: hs + HALF].rearrange(
                        "p (s t) -> p s t", s=NCORES))
            nc.gpsimd.collective_compute(
                "AllToAll", mybir.AluOpType.bypass,
                replica_groups=[list(range(NCORES))],
                ins=[a2a_in[q][:, :]], outs=[a2a_out[q][:, :]])
            ots = p_ots.tile([128, NCORES * TFRAG], BF16, tag="ots",
                             name="ots")
            nc.gpsimd.dma_start(
                out=ots[:].rearrange("p (s t) -> p s t", s=NCORES),
                in_=a2a_out[q][:, :].rearrange("(s p) t -> p s t",
                                               s=NCORES))
            otsT[q] = ots

        def stage_a2a_q(qq):
            """i-tile-sized collective for batch 1 (tokens 2048+qq*512..)."""
            hs = NSEQ + qq * ITILE
            for h in range(2):
                nc.sync.dma_start(
                    out=a2a_in_q[qq][:, :].rearrange(
                        "(s x) t -> x s t", s=NCORES)[h * 64:(h + 1) * 64],
                    in_=outT[h][:, hs: hs + ITILE].rearrange(
                        "p (s t) -> p s t", s=NCORES))
            nc.gpsimd.collective_compute(
                "AllToAll", mybir.AluOpType.bypass,
                replica_groups=[list(range(NCORES))],
                ins=[a2a_in_q[qq][:, :]], outs=[a2a_out_q[qq][:, :]])

        def load_ots_q2(qq0):
            """Pair two 64-token fragments (qq0, qq0+1) into one ots tile."""
            ots = p_ots.tile([128, NCORES * TFRAG], BF16, tag="ots",
                             name="otsq")
            tf = TFRAG // 2
            for k in range(2):
                nc.gpsimd.dma_start(
                    out=ots[:].rearrange("p (s t) -> p s t",
                                         s=NCORES)[:, :, k * tf:(k + 1) * tf],
                    in_=a2a_out_q[qq0 + k][:, :].rearrange(
                        "(s p) t -> p s t", s=NCORES))
            otsT[2 + qq0 // 2] = ots

        def oproj(q):
            # alternate the PSUM pool (ps_mm / ps_ou, both idle in the
            # tail) so consecutive oproj units don't convoy on the 2-buffer
            # ring waiting for the previous unit's bias-adds
            def emit():
                ots = otsT.pop(q)
                pool, tg = [(ps_mm, "mm"), (ps_ou, "ou"),
                            (ps_s, "s"), (ps_ou, "ou")][q]
                y_ps = [pool.tile([128, ITILE], F32, tag=tg, name="yps")
                        for _ in range(2)]
                y_sb = p_y.tile([128, C], F32, tag="y", name="ysb")
                yr0 = q * TFRAG
                # n-outer so the first half's bias-add + y store overlap
                # the second half's matmul chain (shrinks the exposed tail)
                for n in range(2):
                    for s in range(NKC):
                        nc.tensor.matmul(
                            y_ps[n][:],
                            ots[:, s * TFRAG:(s + 1) * TFRAG],
                            wp_sb[:, s * C + n * ITILE: s * C + (n + 1) * ITILE],
                            start=(s == 0), stop=(s == NKC - 1))
                    nc.vector.tensor_add(y_sb[:, n * ITILE:(n + 1) * ITILE],
                                         y_ps[n][:],
                                         bias_bc[:, n * ITILE:(n + 1) * ITILE])
                    # y stores go on sync: on gpsimd they sit behind
                    # CC-gated ots loads and keep y_sb ring buffers alive
                    nc.sync.dma_start(
                        out=y_d[yr0: yr0 + TFRAG, n * ITILE:(n + 1) * ITILE],
                        in_=y_sb[:, n * ITILE:(n + 1) * ITILE])
            return emit

        def drive(gen, fillers):
            """Interleave: one filler unit emitted BEFORE each attention
            j-pair; leftovers drain after the i-tile. None = empty slot."""
            fl = deque(fillers)
            while True:
                if fl:
                    f = fl.popleft()
                    if f is not None:
                        f()
                try:
                    next(gen)
                except StopIteration:
                    break
            while fl:
                f = fl.popleft()
                if f is not None:
                    f()

        def seq(*fns):
            def emit():
                for f in fns:
                    f()
            return emit

        def a2a_unit(q):
            def emit():
                stage_a2a(q)
            return emit

        # ---- emission schedule ----
        for f in qkv_units(0, 0, uu_major=True):
            f()
        for f in vn_units(0, range(0, 8, 2)):
            f()

        drive(attn_pairs(0, 0),
              qkv_units(0, 1, ws=(2, 1)) + vn_units(0, range(8, 16, 2))
              + qkv_units(0, 1, ws=(0,)))
        if DEBUG_DUMP:
            nc.sync.dma_start(out=dbg_qk[0:64, 0:NSEQ], in_=qz[0][0][0:64, :])
            nc.sync.dma_start(out=dbg_qk[64:128, 0:NSEQ],
                              in_=qz[0][1][64:128, :])
            nc.sync.dma_start(out=dbg_qk[0:64, NSEQ:], in_=kz[0][0][0:64, :])
            nc.sync.dma_start(out=dbg_qk[64:128, NSEQ:],
                              in_=kz[0][1][64:128, :])
            for j in range(NJ):
                nc.sync.dma_start(out=dbg_vn[:, j * 130:j * 130 + 130],
                                  in_=vn[j][:, 0:130])

        drive(attn_pairs(0, 1),
              [make_bias_bc, seq(norm_rcp(0, 0, 0), norm_rcp(0, 0, 1)),
               None, norm_mul(0, 0, 0), norm_mul(0, 0, 1)])
        q10 = qkv_units(1, 0)
        drive(attn_pairs(0, 2),
              [q10[0], seq(norm_rcp(0, 1, 0), norm_rcp(0, 1, 1)), q10[1],
               seq(norm_mul(0, 1, 0), norm_mul(0, 1, 1), a2a_unit(0)),
               q10[2], q10[3], q10[4], q10[5]])
        # vn_units(1, (0,1)) must sit at slot >= 1: the slot-k filler is
        # emitted BEFORE attention pair k, and pair 0 still reads batch-0
        # vn[0..1] -- emitting the overwrite first would reorder the data.
        drive(attn_pairs(0, 3),
              [seq(norm_rcp(0, 2, 0), norm_rcp(0, 2, 1))]
              + vn_units(1, range(0, 4, 2))
              + [seq(norm_mul(0, 2, 0), norm_mul(0, 2, 1))]
              + vn_units(1, range(4, 8, 2)))
        qk11 = qkv_units(1, 1, ws=(2, 1))
        vn1hi = vn_units(1, range(8, 16, 2))
        q11q = qkv_units(1, 1, ws=(0,))
        drive(attn_pairs(1, 0),
              [seq(norm_rcp(0, 3, 0), norm_rcp(0, 3, 1)),
               qk11[0], qk11[1],
               seq(norm_mul(0, 3, 0), norm_mul(0, 3, 1), a2a_unit(1)),
               seq(qk11[2], vn1hi[0]), seq(qk11[3], vn1hi[1]),
               vn1hi[2], vn1hi[3], q11q[0], q11q[1]])
        drive(attn_pairs(1, 1),
              [seq(norm_rcp(1, 0, 0), norm_rcp(1, 0, 1)), None,
               seq(norm_mul(1, 0, 0), norm_mul(1, 0, 1)),
               lambda: stage_a2a_q(0)])
        drive(attn_pairs(1, 2),
              [seq(norm_rcp(1, 1, 0), norm_rcp(1, 1, 1)), None, None,
               seq(norm_mul(1, 1, 0), norm_mul(1, 1, 1)),
               lambda: (stage_a2a_q(1), load_ots_q2(0))])
        drive(attn_pairs(1, 3),
              [seq(norm_rcp(1, 2, 0), norm_rcp(1, 2, 1)), None, None,
               seq(norm_mul(1, 2, 0), norm_mul(1, 2, 1)),
               lambda: stage_a2a_q(2)])
        norm_mul(1, 3, 0)(); norm_mul(1, 3, 1)()
        if DEBUG_DUMP:
            nc.sync.dma_start(out=dbg_outT[0:64, :], in_=outT[0][:])
            nc.sync.dma_start(out=dbg_outT[64:128, :], in_=outT[1][:])
        stage_a2a_q(3)
        # deferred output projections: 0-2 fill the PE while the final
        # collective is in flight; 3 runs as soon as its data lands.
        # tile_wait_until pins them to the END of the static schedule --
        # otherwise the scheduler interleaves their matmuls into the
        # attention stream (their a2a data is long ready), delaying the
        # last ACTIVATE that gates the final collective by ~13us, and
        # reorders the ots loads ahead of the final collective trigger on
        # the gpsimd queue.  Runtime order is still semaphore-driven.
        with tc.tile_wait_until(1.0):
            load_ots_q2(2)
            oproj(0)()
            oproj(1)()
            oproj(2)()
            oproj(3)()

    nc.compile()
    return nc


_NC = None


def _get_nc():
    global _NC
    if _NC is None:
        _NC = build_program()
    return _NC


def prep_in_maps(x, w_qkv, w_proj, b_proj):
    x = np.asarray(x, dtype=np.float32).reshape(T, C)
    xT_bf = np.ascontiguousarray(x.T).astype(NPBF16)          # [C, T]
    w_qkv = np.asarray(w_qkv, dtype=np.float32)
    w_proj = np.asarray(w_proj, dtype=np.float32)
    b_proj = np.asarray(b_proj, dtype=np.float32)
    wp_bf = np.ascontiguousarray(w_proj).astype(NPBF16)
    bp_bf = b_proj.reshape(1, C).astype(NPBF16)

    q_w, k_w, v_w = w_qkv[:, 0:C], w_qkv[:, C:2 * C], w_qkv[:, 2 * C:3 * C]
    in_maps = []
    for c in range(NCORES):
        hA, hB = 2 * c, 2 * c + 1
        sA, sB = slice(hA * D, (hA + 1) * D), slice(hB * D, (hB + 1) * D)
        wqk_c = np.concatenate([q_w[:, sA], q_w[:, sB], k_w[:, sA], k_w[:, sB]],
                               axis=1).astype(NPBF16)
        wv_c = np.concatenate([v_w[:, sA], v_w[:, sB]], axis=1).astype(NPBF16)
        in_maps.append({"x": xT_bf, "wqk": np.ascontiguousarray(wqk_c),
                        "wv": np.ascontiguousarray(wv_c), "wproj": wp_bf,
                        "bproj": bp_bf})
    return in_maps


def assemble(results):
    y = np.empty((T, C), dtype=np.float32)
    tf = TFRAG // 2
    for c in range(NCORES):
        yc = results[c]["y"]
        # batch 0: two half-batch fragments of 128 tokens
        for q in range(2):
            g0 = q * HALF + c * TFRAG
            y[g0: g0 + TFRAG, :] = yc[q * TFRAG: (q + 1) * TFRAG, :]
        # batch 1: four i-tile fragments of 64 tokens
        for qq in range(4):
            g0 = NSEQ + qq * ITILE + c * tf
            r0 = 2 * TFRAG + qq * tf
            y[g0: g0 + tf, :] = yc[r0: r0 + tf, :]
    return y.reshape(B, NSEQ, C)


def run(in_maps, trace=False):
    nc = _get_nc()
    return run_bass_kernel_spmd(nc, in_maps, core_ids=list(range(NCORES)),
                                trace=trace)


def kernel(x, w_qkv, w_proj, b_proj):
    res = run(prep_in_maps(x, w_qkv, w_proj, b_proj))
    return assemble(res.results)


# revision 61
# speedup vs baseline: 1.0445x; 1.0021x over previous
"""Multi-head attention (B=2, N=2048, C=1024, H=16) on 8 Trainium2 NeuronCores.

Sharding: tensor-parallel over heads (2 heads/core) for qkv-proj + attention;
all-to-all of the attention output (pipelined under attention), then each
core runs the output projection over the full channel dim for its token
slices.  Host concatenates slices.

Changes vs the 317us baseline (this version: ~285us):
  - x is pre-transposed on the HOST (numpy) so x^T loads are plain 2D DMAs
    in u-major order spread across the three DMA-capable queues
    (sync/scalar/gpsimd) -- kills the serialized DMA-xbar transpose chain
    that idled the PE for the first 30us.  (Do NOT emit dma_start_transpose
    mid-schedule: Tile's transpose/collective serialization corrupts.)
  - softmax 1/den via exp(-ln(den)) on ScalarE.  Ln+Exp share one
    activation table set; get_activation_tables is patched (order-
    preserving) to force both onto natural_log_exp_and_others, otherwise
    the table-load pass thrashes ~2.7us loads between two sets.  The
    Ln/Exp pair runs as a filler in the NEXT drive so it doesn't wedge
    between two i-tiles' exp streams (frees the DVE of the 3.3us
    iterative-divide reciprocal entirely).
  - exchanges: batch 0 as two 256KB half-batch all-to-alls (deep-hidden);
    batch 1 as four 128KB i-tile-sized ones so the only tail-exposed
    collective is 128KB.  A dummy 64B collective at startup absorbs the
    ~11.5us first-collective warmup.  All CC-gated DMAs (ots loads) live
    on the gpsimd DGE queue in trigger order where their waits are
    harmless; a CC-gated DMA at the head of the sync queue blocks the
    norm-critical broadcasts (head-of-line), and on the scalar queue it
    stalls the exp pipeline.
  - all four output projections are deferred to the tail (pinned there
    via tile_wait_until -- the scheduler otherwise interleaves them into
    the attention stream, delaying the last ACTIVATE that gates the final
    collective by ~13us); 0-2 execute under the final collective, 3 right
    after it, alternating PSUM pools (mm/ou/s) to avoid ring convoys.

Per-core structure (heads A=2c, B=2c+1):
  x^T  [c_in, tok]       host-pretransposed, plain DMA
  q/k  [128, tok]        rows 0-63 head A dims, 64-127 head B dims (packed)
  S^T  [128, 1024]       per j-chunk: head A cols 0:512, head B 512:1024
  exp  one ACTIVATE per j-chunk over [128, 1024] PSUM (both heads);
                         S PSUM tiles ping-pong (bufs=2)
  out_u^T [65, i] = [v|1].T @ expS  (row 64 = softmax denominator)
  normalize: ScalarE exp(-ln(den)) + DMA broadcast + DVE multiply
"""

import numpy as np
import ml_dtypes
from collections import deque
from contextlib import ExitStack

import concourse.bass as bass
import concourse.tile as tile
from concourse import bacc, mybir
from concourse import hw_specs as _hw_specs
from concourse.bass_utils import run_bass_kernel_spmd
from concourse.masks import make_identity

# The act-table-load pass picks the FIRST table set containing each
# activation function, which puts Exp in `exp_and_others` and Ln in
# `natural_log` and thrashes ~2.7us table loads between them.  Both live
# together in `natural_log_exp_and_others`; steer the pass there by hiding
# Exp/Ln from every other set.  Entry ORDER (= act_func_set_id indexing)
# is preserved, so walrus still resolves the chosen set correctly.
_orig_gat = getattr(_hw_specs, "_bass_kernel_orig_gat", None) \
    or _hw_specs.get_activation_tables
_hw_specs._bass_kernel_orig_gat = _orig_gat


def _patched_gat(module_arch):
    tabs = _orig_gat(module_arch)
    T = mybir.ActivationFunctionType
    for name, fns in tabs.items():
        if name != "natural_log_exp_and_others":
            fns.discard(T.Exp)
            fns.discard(T.Ln)
    return tabs


_hw_specs.get_activation_tables = _patched_gat
bacc.get_activation_tables = _patched_gat

BF16 = mybir.dt.bfloat16
F32 = mybir.dt.float32
EXP = mybir.ActivationFunctionType.Exp
LN = mybir.ActivationFunctionType.Ln
NPBF16 = ml_dtypes.bfloat16

NCORES = 8
B, NSEQ, C, H, D = 2, 2048, 1024, 16, 64
T = B * NSEQ                 # 4096 flattened tokens
SCALE = D ** -0.5            # folded into the exp activation
NKC = C // 128               # 8 contraction chunks
ITILE = 512                  # query tile (free dim of S^T)
NI = NSEQ // ITILE           # 4 i-tiles per batch
NJ = NSEQ // 128             # 16 key chunks per batch
HALF = 1024                  # tokens per all-to-all (half batch)
TFRAG = HALF // NCORES       # 128 tokens per core per all-to-all
TSL = B * NSEQ // NCORES     # 512 output tokens per core

import os
DEBUG_DUMP = os.environ.get("KDBG", "") == "1"


def build_program():
    nc = bacc.Bacc("TRN2", target_bir_lowering=False, debug=False,
                   num_devices=NCORES)

    # x arrives pre-transposed from the host: [C, T]
    xT_d = nc.dram_tensor("x", [C, T], BF16, kind="ExternalInput")
    wqk_d = nc.dram_tensor("wqk", [C, 256], BF16, kind="ExternalInput")
    wv_d = nc.dram_tensor("wv", [C, 128], BF16, kind="ExternalInput")
    wp_d = nc.dram_tensor("wproj", [C, C], BF16, kind="ExternalInput")
    bp_d = nc.dram_tensor("bproj", [1, C], BF16, kind="ExternalInput")
    y_d = nc.dram_tensor("y", [TSL, C], F32, kind="ExternalOutput")

    # batch 0 exchanged as two half-batch collectives (deep-hidden); batch 1
    # as four i-tile-sized ones so the last exposed collective is only 128KB
    a2a_in = [nc.dram_tensor(f"a2a_in{q}", [NCORES * 128, TFRAG], BF16)
              for q in range(2)]
    a2a_out = [nc.dram_tensor(f"a2a_out{q}", [NCORES * 128, TFRAG], BF16)
               for q in range(2)]
    a2a_in_q = [nc.dram_tensor(f"a2a_inq{q}", [NCORES * 128, TFRAG // 2],
                               BF16) for q in range(4)]
    a2a_out_q = [nc.dram_tensor(f"a2a_outq{q}", [NCORES * 128, TFRAG // 2],
                                BF16) for q in range(4)]
    warm_in = nc.dram_tensor("warm_in", [NCORES, 4], BF16)
    warm_out = nc.dram_tensor("warm_out", [NCORES, 4], BF16)
    rcp_d = [nc.dram_tensor(f"rcp_d{s}", [1, ITILE], BF16) for s in range(4)]

    if DEBUG_DUMP:
        dbg_qk = nc.dram_tensor("dbg_qk", [128, 2 * NSEQ], BF16,
                                kind="ExternalOutput")
        dbg_vn = nc.dram_tensor("dbg_vn", [128, NJ * 130], BF16,
                                kind="ExternalOutput")
        dbg_ouc = nc.dram_tensor("dbg_ouc", [65, 8 * ITILE], F32,
                                 kind="ExternalOutput")
        dbg_outT = nc.dram_tensor("dbg_outT", [128, T], BF16,
                                  kind="ExternalOutput")

    with tile.TileContext(nc) as tc, ExitStack() as ctx:
        ep = ctx.enter_context

        consts = ep(tc.tile_pool(name="consts", bufs=1))
        p_exp = ep(tc.tile_pool(name="exps", bufs=4))
        p_ouc = ep(tc.tile_pool(name="ouc", bufs=6))
        p_small = ep(tc.tile_pool(name="small", bufs=4))
        p_ots = ep(tc.tile_pool(name="ots", bufs=4))
        p_y = ep(tc.tile_pool(name="ysb", bufs=2))
        ps_s = ep(tc.tile_pool(name="pss", bufs=2, space="PSUM"))
        ps_ou = ep(tc.tile_pool(name="psou", bufs=2, space="PSUM"))
        ps_mm = ep(tc.tile_pool(name="psmm", bufs=2, space="PSUM"))

        # ---- weights / constants to SBUF ----
        wqk_sb = consts.tile([128, NKC * 256], BF16, name="wqk_sb")
        wv_sb = consts.tile([128, NKC * 128], BF16, name="wv_sb")
        wp_sb = consts.tile([128, NKC * C], BF16, name="wp_sb")
        bp_sb = consts.tile([1, C], BF16, name="bp_sb")

        # x^T: one tile per batch, layout [:, c*2048 + t]
        xt = [consts.tile([128, NKC * NSEQ], BF16, name=f"xt{b}")
              for b in range(B)]

        def load_xu(b, u, cs, eng):
            """Plain 2D DMA of one u-slice (512 tokens) of x^T chunks."""
            for c in cs:
                eng.dma_start(
                    out=xt[b][:, c * NSEQ + u * ITILE:
                              c * NSEQ + (u + 1) * ITILE],
                    in_=xT_d[c * 128:(c + 1) * 128,
                             b * NSEQ + u * ITILE: b * NSEQ + (u + 1) * ITILE])

        def load_wqk(c0, c1, eng):
            eng.dma_start(
                out=wqk_sb[:].rearrange("p (c n) -> p c n",
                                        c=NKC)[:, c0:c1],
                in_=wqk_d[c0 * 128:c1 * 128, :].rearrange(
                    "(c p) n -> p c n", p=128))

        # Startup DMA plan: u-major x slices so each qkv unit's 8 c-chunks
        # arrive together, spread across the three DMA-capable queues
        # (sync/SP, scalar, gpsimd) in first-use order.
        load_wqk(0, 4, nc.sync)
        load_wqk(4, 8, nc.gpsimd)
        # Dummy collective: absorbs the ~11.5us first-collective warmup
        # delay during the qkv phase so a2a(0) starts promptly.
        nc.gpsimd.collective_compute(
            "AllToAll", mybir.AluOpType.bypass,
            replica_groups=[list(range(NCORES))],
            ins=[warm_in[:, :]], outs=[warm_out[:, :]])
        nc.gpsimd.dma_start(
            out=wv_sb[:].rearrange("p (c n) -> p c n", c=NKC),
            in_=wv_d[:, :].rearrange("(c p) n -> p c n", p=128))
        for u in range(4):
            load_xu(0, u, range(0, 4), nc.sync)
            load_xu(0, u, range(4, 8), nc.scalar)
        nc.sync.dma_start(out=bp_sb[:], in_=bp_d[0:1, :])
        for u in range(4):
            load_xu(1, u, range(0, 8), nc.gpsimd)
        nc.sync.dma_start(
            out=wp_sb[:].rearrange("p (c n) -> p c n", c=NKC),
            in_=wp_d[:, :].rearrange("(c p) n -> p c n", p=128))

        ident = consts.tile([128, 128], BF16, name="ident")
        make_identity(nc, ident[:])
        onesc = consts.tile([1, 128], BF16, name="onesc")
        nc.vector.memset(onesc[:], 1.0)

        # bias broadcast [128, C] f32, computed once via 1x128 outer product
        bias_bc = consts.tile([128, C], F32, name="bias_bc")

        def make_bias_bc():
            for n in range(2):
                bps = ps_mm.tile([128, ITILE], F32, tag="mm", name="bps")
                nc.tensor.matmul(bps[:], onesc[:],
                                 bp_sb[:, n * ITILE:(n + 1) * ITILE],
                                 start=True, stop=True)
                nc.vector.tensor_copy(bias_bc[:, n * ITILE:(n + 1) * ITILE],
                                      bps[:])

        # ---- persistent per-batch / per-chunk state ----
        qz = [[consts.tile([128, NSEQ], BF16, name=f"qz{b}{h}")
               for h in range(2)] for b in range(B)]
        kz = [[consts.tile([128, NSEQ], BF16, name=f"kz{b}{h}")
               for h in range(2)] for b in range(B)]
        for b in range(B):
            nc.vector.memset(qz[b][0][64:128, :], 0.0)
            nc.vector.memset(kz[b][0][64:128, :], 0.0)
            nc.vector.memset(qz[b][1][0:64, :], 0.0)
            nc.vector.memset(kz[b][1][0:64, :], 0.0)
        vT = [consts.tile([128, NSEQ], BF16, name=f"vT{b}") for b in range(B)]
        # vn[j]: [v_A(64) | 1 | v_B(64) | 1 | zeros(63)]; constants written once
        vn = [consts.tile([128, 193], BF16, name=f"vn{j}") for j in range(NJ)]
        for j in range(NJ):
            nc.vector.memset(vn[j][:, 64:65], 1.0)
            nc.vector.memset(vn[j][:, 129:130], 1.0)
            nc.vector.memset(vn[j][:, 130:193], 0.0)
        # normalized attention output, per head (partitions 0-63)
        outT = [consts.tile([64, T], BF16, name=f"outT{h}") for h in range(2)]

        def xts(b, u, c):
            return xt[b][:, c * NSEQ + u * ITILE: c * NSEQ + (u + 1) * ITILE]

        # ---- qkv projection: one (w, u) unit = 8 matmuls + 1 evac ----
        def qkv_unit(b, tp, w, uu):
            def emit():
                u = 2 * tp + uu
                usl = slice(u * ITILE, (u + 1) * ITILE)
                pst = ps_mm.tile([128, ITILE], F32, tag="mm", name="pst")
                for c in range(NKC):
                    if w < 2:
                        lhsT = wqk_sb[:, c * 256 + w * 128:
                                      c * 256 + (w + 1) * 128]
                    else:
                        lhsT = wv_sb[:, c * 128:(c + 1) * 128]
                    nc.tensor.matmul(pst[:], lhsT, xts(b, u, c),
                                     start=(c == 0), stop=(c == NKC - 1))
                if w == 2:
                    nc.vector.tensor_copy(vT[b][:, usl], pst[:])
                else:
                    dst = (qz, kz)[w][b]
                    nc.vector.tensor_copy(dst[0][0:64, usl], pst[0:64, :])
                    nc.vector.tensor_copy(dst[1][64:128, usl],
                                          pst[64:128, :])
            return emit

        def qkv_units(b, tp, ws=(0, 1, 2), uu_major=False):
            if uu_major:
                return [qkv_unit(b, tp, w, uu) for uu in range(2) for w in ws]
            return [qkv_unit(b, tp, w, uu) for w in ws for uu in range(2)]

        # ---- vn construction: one unit = 2 transposes + 4 copies ----
        # (PE transposes: DMA-xbar transposes mid-schedule corrupt results,
        # Tile's transpose/collective serialization cannot handle them)
        def vn_unit(b, tcn0):
            def emit():
                for tcn in (tcn0, tcn0 + 1):
                    vtr = ps_mm.tile([128, 128], BF16, tag="mm", name="vtr")
                    nc.tensor.transpose(vtr[:],
                                        vT[b][:, tcn * 128:(tcn + 1) * 128],
                                        ident[:])
                    nc.vector.tensor_copy(vn[tcn][:, 0:64], vtr[:, 0:64])
                    nc.vector.tensor_copy(vn[tcn][:, 65:129], vtr[:, 64:128])
            return emit

        def vn_units(b, tcns):
            return [vn_unit(b, t0) for t0 in tcns]

        # ---- attention ----
        outUc = {}

        def attn_pairs(b, i, sub=None):
            """Generator: one yield per j-chunk pair (8 per i-tile).
            sub=0/1 processes only a 256-query half of the i-tile (used to
            shrink the final exchanged fragment)."""
            if sub is None:
                q0, W = i * ITILE, ITILE
            else:
                q0, W = i * ITILE + sub * (ITILE // 2), ITILE // 2
            isl = slice(q0, q0 + W)
            outu = [ps_ou.tile([128, W], F32, tag="ou", name="outu")
                    for _ in range(2)]
            for g in range(NJ // 2):
                sts = []
                for jj in (2 * g, 2 * g + 1):
                    s_t = ps_s.tile([128, 2 * W], F32, tag="s", name="s_t")
                    for h in range(2):
                        nc.tensor.matmul(
                            s_t[:, h * W:(h + 1) * W],
                            kz[b][h][:, jj * 128:(jj + 1) * 128],
                            qz[b][h][:, isl],
                            start=True, stop=True)
                    sts.append(s_t)
                exs = []
                for k in range(2):
                    ex = p_exp.tile([128, 2 * W], BF16, tag="ex", name="ex")
                    nc.scalar.activation(ex[:], sts[k][:], EXP, scale=SCALE)
                    exs.append(ex)
                for k, jj in enumerate((2 * g, 2 * g + 1)):
                    for h in range(2):
                        nc.tensor.matmul(
                            outu[h][:],
                            vn[jj][:, h * 65: h * 65 + 128],
                            exs[k][:, h * W:(h + 1) * W],
                            start=(jj == 0), stop=(jj == NJ - 1))
                yield
            last = (b, i) == (B - 1, NI - 1) and sub in (None, 1)
            for h in range(2):
                slot = (b * 8 + i * 2 + h) % 4
                if last and not DEBUG_DUMP:
                    # last sub-tile: run the whole reciprocal chain inline
                    # (tail latency) and read PSUM directly -- nothing
                    # reuses this PSUM afterwards, no evac copy needed.
                    lnt = p_small.tile([65, W], F32, tag="rcp", name="lnt")
                    nc.scalar.activation(lnt[64:65, :], outu[h][64:65, :],
                                         LN)
                    rcpb = p_small.tile([65, W], BF16, tag="rcpb",
                                        name="rcpb")
                    nc.scalar.activation(rcpb[64:65, :], lnt[64:65, :],
                                         EXP, scale=-1.0)
                    nc.sync.dma_start(out=rcp_d[slot][0:1, 0:W],
                                      in_=rcpb[64:65, :])
                    outUc[(b, i, h, sub)] = (outu[h], slot, q0, W)
                    continue
                ouc = p_ouc.tile([65, W], F32, tag="ouc", name="ouc")
                nc.vector.tensor_copy(ouc[:], outu[h][0:65, :])
                outUc[(b, i, h, sub)] = (ouc, slot, q0, W)
                if DEBUG_DUMP and b == 0:
                    sl = (i * 2 + h) * ITILE
                    nc.sync.dma_start(out=dbg_ouc[:, sl:sl + W],
                                      in_=ouc[:])

        def norm_rcp(b, i, h, sub=None):
            """1/den = exp(-ln(den)) on ScalarE from the ouc copy.  Emitted
            as a filler in the NEXT drive so the two extra ScalarE ops don't
            wedge between two i-tiles' exp streams and stall the AV refill
            (Ln and Exp share one table set -- no table switching)."""
            def emit():
                ouc, slot, q0, W = outUc[(b, i, h, sub)]
                lnt = p_small.tile([65, W], F32, tag="rcp", name="lnt")
                nc.scalar.activation(lnt[64:65, :], ouc[64:65, :], LN)
                rcpb = p_small.tile([65, W], BF16, tag="rcpb", name="rcpb")
                nc.scalar.activation(rcpb[64:65, :], lnt[64:65, :], EXP,
                                     scale=-1.0)
                nc.sync.dma_start(out=rcp_d[slot][0:1, 0:W],
                                  in_=rcpb[64:65, :])
            return emit

        def norm_mul(b, i, h, sub=None):
            """DMA broadcast of 1/den + DVE multiply; scheduled a few slots
            after the i-tile so the rcp_d write latency is hidden."""
            def emit():
                t0 = b * NSEQ
                ouc, slot, q0, W = outUc.pop((b, i, h, sub))
                bc_sb = p_small.tile([64, W], BF16, tag="bc", name="bcsb")
                nc.sync.dma_start(
                    out=bc_sb[:],
                    in_=rcp_d[slot][0:1, 0:W].to_broadcast((64, W)))
                nc.vector.tensor_mul(
                    outT[h][0:64, t0 + q0: t0 + q0 + W],
                    ouc[0:64, :], bc_sb[:])
            return emit

        # All CC-gated DMAs (ots loads, y stores) live on the gpsimd DGE
        # queue in natural order (trigger q -> ots q -> trigger q+1 ...):
        # their semaphore waits are harmless there because everything queued
        # behind them is gated even later.  On sync they head-of-line-block
        # the norm broadcasts; on scalar they stall the exp pipeline.
        otsT = {}

        def stage_a2a(q):
            """Half-batch collective for batch 0 (tokens q*1024..)."""
            hs = q * HALF
            for h in range(2):
                nc.sync.dma_start(
                    out=a2a_in[q][:, :].rearrange(
                        "(s x) t -> x s t", s=NCORES)[h * 64:(h + 1) * 64],
                    in_=outT[h][:, hs: hs + HALF].rearrange(
                        "p (s t) -> p s t", s=NCORES))
            nc.gpsimd.collective_compute(
                "AllToAll", mybir.AluOpType.bypass,
                replica_groups=[list(range(NCORES))],
                ins=[a2a_in[q][:, :]], outs=[a2a_out[q][:, :]])
            ots = p_ots.tile([128, NCORES * TFRAG], BF16, tag="ots",
                             name="ots")
            nc.gpsimd.dma_start(
                out=ots[:].rearrange("p (s t) -> p s t", s=NCORES),
                in_=a2a_out[q][:, :].rearrange("(s p) t -> p s t",
                                               s=NCORES))
            otsT[q] = ots

        def stage_a2a_q(qq):
            """i-tile-sized collective for batch 1 (tokens 2048+qq*512..)."""
            hs = NSEQ + qq * ITILE
            for h in range(2):
                nc.sync.dma_start(
                    out=a2a_in_q[qq][:, :].rearrange(
                        "(s x) t -> x s t", s=NCORES)[h * 64:(h + 1) * 64],
                    in_=outT[h][:, hs: hs + ITILE].rearrange(
                        "p (s t) -> p s t", s=NCORES))
            nc.gpsimd.collective_compute(
                "AllToAll", mybir.AluOpType.bypass,
                replica_groups=[list(range(NCORES))],
                ins=[a2a_in_q[qq][:, :]], outs=[a2a_out_q[qq][:, :]])

        def load_ots_q2(qq0):
            """Pair two 64-token fragments (qq0, qq0+1) into one ots tile."""
            ots = p_ots.tile([128, NCORES * TFRAG], BF16, tag="ots",
                             name="otsq")
            tf = TFRAG // 2
            for k in range(2):
                nc.gpsimd.dma_start(
                    out=ots[:].rearrange("p (s t) -> p s t",
                                         s=NCORES)[:, :, k * tf:(k + 1) * tf],
                    in_=a2a_out_q[qq0 + k][:, :].rearrange(
                        "(s p) t -> p s t", s=NCORES))
            otsT[2 + qq0 // 2] = ots

        def oproj(q):
            # alternate the PSUM pool (ps_mm / ps_ou, both idle in the
            # tail) so consecutive oproj units don't convoy on the 2-buffer
            # ring waiting for the previous unit's bias-adds
            def emit():
                ots = otsT.pop(q)
                pool, tg = [(ps_mm, "mm"), (ps_ou, "ou"),
                            (ps_s, "s"), (ps_ou, "ou")][q]
                y_ps = [pool.tile([128, ITILE], F32, tag=tg, name="yps")
                        for _ in range(2)]
                y_sb = p_y.tile([128, C], F32, tag="y", name="ysb")
                yr0 = q * TFRAG
                # n-outer so the first half's bias-add + y store overlap
                # the second half's matmul chain (shrinks the exposed tail)
                for n in range(2):
                    for s in range(NKC):
                        nc.tensor.matmul(
                            y_ps[n][:],
                            ots[:, s * TFRAG:(s + 1) * TFRAG],
                            wp_sb[:, s * C + n * ITILE: s * C + (n + 1) * ITILE],
                            start=(s == 0), stop=(s == NKC - 1))
                    nc.vector.tensor_add(y_sb[:, n * ITILE:(n + 1) * ITILE],
                                         y_ps[n][:],
                                         bias_bc[:, n * ITILE:(n + 1) * ITILE])
                    # y stores go on sync: on gpsimd they sit behind
                    # CC-gated ots loads and keep y_sb ring buffers alive
                    nc.sync.dma_start(
                        out=y_d[yr0: yr0 + TFRAG, n * ITILE:(n + 1) * ITILE],
                        in_=y_sb[:, n * ITILE:(n + 1) * ITILE])
            return emit

        def drive(gen, fillers):
            """Interleave: one filler unit emitted BEFORE each attention
            j-pair; leftovers drain after the i-tile. None = empty slot."""
            fl = deque(fillers)
            while True:
                if fl:
                    f = fl.popleft()
                    if f is not None:
                        f()
                try:
                    next(gen)
                except StopIteration:
                    break
            while fl:
                f = fl.popleft()
                if f is not None:
                    f()

        def seq(*fns):
            def emit():
                for f in fns:
                    f()
            return emit

        def a2a_unit(q):
            def emit():
                stage_a2a(q)
            return emit

        # ---- emission schedule ----
        for f in qkv_units(0, 0, uu_major=True):
            f()
        for f in vn_units(0, range(0, 8, 2)):
            f()

        drive(attn_pairs(0, 0),
              qkv_units(0, 1, ws=(2, 1)) + vn_units(0, range(8, 16, 2))
              + qkv_units(0, 1, ws=(0,)))
        if DEBUG_DUMP:
            nc.sync.dma_start(out=dbg_qk[0:64, 0:NSEQ], in_=qz[0][0][0:64, :])
            nc.sync.dma_start(out=dbg_qk[64:128, 0:NSEQ],
                              in_=qz[0][1][64:128, :])
            nc.sync.dma_start(out=dbg_qk[0:64, NSEQ:], in_=kz[0][0][0:64, :])
            nc.sync.dma_start(out=dbg_qk[64:128, NSEQ:],
                              in_=kz[0][1][64:128, :])
            for j in range(NJ):
                nc.sync.dma_start(out=dbg_vn[:, j * 130:j * 130 + 130],
                                  in_=vn[j][:, 0:130])

        drive(attn_pairs(0, 1),
              [make_bias_bc, seq(norm_rcp(0, 0, 0), norm_rcp(0, 0, 1)),
               None, norm_mul(0, 0, 0), norm_mul(0, 0, 1)])
        q10 = qkv_units(1, 0)
        drive(attn_pairs(0, 2),
              [q10[0], seq(norm_rcp(0, 1, 0), norm_rcp(0, 1, 1)), q10[1],
               seq(norm_mul(0, 1, 0), norm_mul(0, 1, 1), a2a_unit(0)),
               q10[2], q10[3], q10[4], q10[5]])
        # vn_units(1, (0,1)) must sit at slot >= 1: the slot-k filler is
        # emitted BEFORE attention pair k, and pair 0 still reads batch-0
        # vn[0..1] -- emitting the overwrite first would reorder the data.
        drive(attn_pairs(0, 3),
              [seq(norm_rcp(0, 2, 0), norm_rcp(0, 2, 1))]
              + vn_units(1, range(0, 4, 2))
              + [seq(norm_mul(0, 2, 0), norm_mul(0, 2, 1))]
              + vn_units(1, range(4, 8, 2)))
        qk11 = qkv_units(1, 1, ws=(2, 1))
        vn1hi = vn_units(1, range(8, 16, 2))
        q11q = qkv_units(1, 1, ws=(0,))
        drive(attn_pairs(1, 0),
              [seq(norm_rcp(0, 3, 0), norm_rcp(0, 3, 1)),
               qk11[0], qk11[1],
               seq(norm_mul(0, 3, 0), norm_mul(0, 3, 1), a2a_unit(1)),
               seq(qk11[2], vn1hi[0]), seq(qk11[3], vn1hi[1]),
               vn1hi[2], vn1hi[3], q11q[0], q11q[1]])
        drive(attn_pairs(1, 1),
              [seq(norm_rcp(1, 0, 0), norm_rcp(1, 0, 1)), None,
               seq(norm_mul(1, 0, 0), norm_mul(1, 0, 1)),
               lambda: stage_a2a_q(0)])
        drive(attn_pairs(1, 2),
              [seq(norm_rcp(1, 1, 0), norm_rcp(1, 1, 1)), None, None,
               seq(norm_mul(1, 1, 0), norm_mul(1, 1, 1)),
               lambda: (stage_a2a_q(1), load_ots_q2(0))])
        drive(attn_pairs(1, 3),
              [seq(norm_rcp(1, 2, 0), norm_rcp(1, 2, 1)), None, None,
               seq(norm_mul(1, 2, 0), norm_mul(1, 2, 1)),
               lambda: stage_a2a_q(2)])
        norm_mul(1, 3, 0)(); norm_mul(1, 3, 1)()
        if DEBUG_DUMP:
            nc.sync.dma_start(out=dbg_outT[0:64, :], in_=outT[0][:])
            nc.sync.dma_start(out=dbg_outT[64:128, :], in_=outT[1][:])
        stage_a2a_q(3)
        # deferred output projections: 0-2 fill the PE while the final
        # collective is in flight; 3 runs as soon as its data lands.
        # tile_wait_until pins them to the END of the static schedule --
        # otherwise the scheduler interleaves their matmuls into the
        # attention stream (their a2a data is long ready), delaying the
        # last ACTIVATE that gates the final collective by ~13us, and
        # reorders the ots loads ahead of the final collective trigger on
        # the gpsimd queue.  Runtime order is still semaphore-driven.
        with tc.tile_wait_until(1.0):
            load_ots_q2(2)
            oproj(0)()
            oproj(1)()
            oproj(2)()
            oproj(3)()

    nc.compile()
    return nc


_NC = None


def _get_nc():
    global _NC
    if _NC is None:
        _NC = build_program()
    return _NC


def prep_in_maps(x, w_qkv, w_proj, b_proj):
    x = np.asarray(x, dtype=np.float32).reshape(T, C)
    xT_bf = np.ascontiguousarray(x.T).astype(NPBF16)          # [C, T]
    w_qkv = np.asarray(w_qkv, dtype=np.float32)
    w_proj = np.asarray(w_proj, dtype=np.float32)
    b_proj = np.asarray(b_proj, dtype=np.float32)
    wp_bf = np.ascontiguousarray(w_proj).astype(NPBF16)
    bp_bf = b_proj.reshape(1, C).astype(NPBF16)

    q_w, k_w, v_w = w_qkv[:, 0:C], w_qkv[:, C:2 * C], w_qkv[:, 2 * C:3 * C]
    in_maps = []
    for c in range(NCORES):
        hA, hB = 2 * c, 2 * c + 1
        sA, sB = slice(hA * D, (hA + 1) * D), slice(hB * D, (hB + 1) * D)
        wqk_c = np.concatenate([q_w[:, sA], q_w[:, sB], k_w[:, sA], k_w[:, sB]],
                               axis=1).astype(NPBF16)
        wv_c = np.concatenate([v_w[:, sA], v_w[:, sB]], axis=1).astype(NPBF16)
        in_maps.append({"x": xT_bf, "wqk": np.ascontiguousarray(wqk_c),
                        "wv": np.ascontiguousarray(wv_c), "wproj": wp_bf,
                        "bproj": bp_bf})
    return in_maps


def assemble(results):
    y = np.empty((T, C), dtype=np.float32)
    tf = TFRAG // 2
    for c in range(NCORES):
        yc = results[c]["y"]
        # batch 0: two half-batch fragments of 128 tokens
        for q in range(2):
            g0 = q * HALF + c * TFRAG
            y[g0: g0 + TFRAG, :] = yc[q * TFRAG: (q + 1) * TFRAG, :]
        # batch 1: four i-tile fragments of 64 tokens
        for qq in range(4):
            g0 = NSEQ + qq * ITILE + c * tf
            r0 = 2 * TFRAG + qq * tf
            y[g0: g0 + tf, :] = yc[r0: r0 + tf, :]
    return y.reshape(B, NSEQ, C)


def run(in_maps, trace=False):
    nc = _get_nc()
    return run_bass_kernel_spmd(nc, in_maps, core_ids=list(range(NCORES)),
                                trace=trace)


def kernel(x, w_qkv, w_proj, b_proj):
    res = run(prep_in_maps(x, w_qkv, w_proj, b_proj))
    return assemble(res.results)


# revision 62
# speedup vs baseline: 1.0483x; 1.0037x over previous
"""Multi-head attention (B=2, N=2048, C=1024, H=16) on 8 Trainium2 NeuronCores.

Sharding: tensor-parallel over heads (2 heads/core) for qkv-proj + attention;
all-to-all of the attention output (pipelined under attention), then each
core runs the output projection over the full channel dim for its token
slices.  Host concatenates slices.

Changes vs the 317us baseline (this version: ~285us):
  - x is pre-transposed on the HOST (numpy) so x^T loads are plain 2D DMAs
    in u-major order spread across the three DMA-capable queues
    (sync/scalar/gpsimd) -- kills the serialized DMA-xbar transpose chain
    that idled the PE for the first 30us.  (Do NOT emit dma_start_transpose
    mid-schedule: Tile's transpose/collective serialization corrupts.)
  - softmax 1/den via exp(-ln(den)) on ScalarE.  Ln+Exp share one
    activation table set; get_activation_tables is patched (order-
    preserving) to force both onto natural_log_exp_and_others, otherwise
    the table-load pass thrashes ~2.7us loads between two sets.  The
    Ln/Exp pair runs as a filler in the NEXT drive so it doesn't wedge
    between two i-tiles' exp streams (frees the DVE of the 3.3us
    iterative-divide reciprocal entirely).
  - exchanges: batch 0 as two 256KB half-batch all-to-alls (deep-hidden);
    batch 1 as four 128KB i-tile-sized ones so the only tail-exposed
    collective is 128KB.  A dummy 64B collective at startup absorbs the
    ~11.5us first-collective warmup.  All CC-gated DMAs (ots loads) live
    on the gpsimd DGE queue in trigger order where their waits are
    harmless; a CC-gated DMA at the head of the sync queue blocks the
    norm-critical broadcasts (head-of-line), and on the scalar queue it
    stalls the exp pipeline.
  - all four output projections are deferred to the tail (pinned there
    via tile_wait_until -- the scheduler otherwise interleaves them into
    the attention stream, delaying the last ACTIVATE that gates the final
    collective by ~13us); 0-2 execute under the final collective, 3 right
    after it, alternating PSUM pools (mm/ou/s) to avoid ring convoys.

Per-core structure (heads A=2c, B=2c+1):
  x^T  [c_in, tok]       host-pretransposed, plain DMA
  q/k  [128, tok]        rows 0-63 head A dims, 64-127 head B dims (packed)
  S^T  [128, 1024]       per j-chunk: head A cols 0:512, head B 512:1024
  exp  one ACTIVATE per j-chunk over [128, 1024] PSUM (both heads);
                         S PSUM tiles ping-pong (bufs=2)
  out_u^T [65, i] = [v|1].T @ expS  (row 64 = softmax denominator)
  normalize: ScalarE exp(-ln(den)) + DMA broadcast + DVE multiply
"""

import numpy as np
import ml_dtypes
from collections import deque
from contextlib import ExitStack

import concourse.bass as bass
import concourse.tile as tile
from concourse import bacc, mybir
from concourse import hw_specs as _hw_specs
from concourse.bass_utils import run_bass_kernel_spmd
from concourse.masks import make_identity

# The act-table-load pass picks the FIRST table set containing each
# activation function, which puts Exp in `exp_and_others` and Ln in
# `natural_log` and thrashes ~2.7us table loads between them.  Both live
# together in `natural_log_exp_and_others`; steer the pass there by hiding
# Exp/Ln from every other set.  Entry ORDER (= act_func_set_id indexing)
# is preserved, so walrus still resolves the chosen set correctly.
_orig_gat = getattr(_hw_specs, "_bass_kernel_orig_gat", None) \
    or _hw_specs.get_activation_tables
_hw_specs._bass_kernel_orig_gat = _orig_gat


def _patched_gat(module_arch):
    tabs = _orig_gat(module_arch)
    T = mybir.ActivationFunctionType
    for name, fns in tabs.items():
        if name != "natural_log_exp_and_others":
            fns.discard(T.Exp)
            fns.discard(T.Ln)
    return tabs


_hw_specs.get_activation_tables = _patched_gat
bacc.get_activation_tables = _patched_gat

BF16 = mybir.dt.bfloat16
F32 = mybir.dt.float32
EXP = mybir.ActivationFunctionType.Exp
LN = mybir.ActivationFunctionType.Ln
NPBF16 = ml_dtypes.bfloat16

NCORES = 8
B, NSEQ, C, H, D = 2, 2048, 1024, 16, 64
T = B * NSEQ                 # 4096 flattened tokens
SCALE = D ** -0.5            # folded into the exp activation
NKC = C // 128               # 8 contraction chunks
ITILE = 512                  # query tile (free dim of S^T)
NI = NSEQ // ITILE           # 4 i-tiles per batch
NJ = NSEQ // 128             # 16 key chunks per batch
HALF = 1024                  # tokens per all-to-all (half batch)
TFRAG = HALF // NCORES       # 128 tokens per core per all-to-all
TSL = B * NSEQ // NCORES     # 512 output tokens per core

import os
DEBUG_DUMP = os.environ.get("KDBG", "") == "1"


def build_program():
    nc = bacc.Bacc("TRN2", target_bir_lowering=False, debug=False,
                   num_devices=NCORES)

    # x arrives pre-transposed from the host: [C, T]
    xT_d = nc.dram_tensor("x", [C, T], BF16, kind="ExternalInput")
    wqk_d = nc.dram_tensor("wqk", [C, 256], BF16, kind="ExternalInput")
    wv_d = nc.dram_tensor("wv", [C, 128], BF16, kind="ExternalInput")
    wp_d = nc.dram_tensor("wproj", [C, C], BF16, kind="ExternalInput")
    bp_d = nc.dram_tensor("bproj", [1, C], BF16, kind="ExternalInput")
    y_d = nc.dram_tensor("y", [TSL, C], F32, kind="ExternalOutput")

    # batch 0 exchanged as two half-batch collectives (deep-hidden); batch 1
    # as four i-tile-sized ones so the last exposed collective is only 128KB
    a2a_in = [nc.dram_tensor(f"a2a_in{q}", [NCORES * 128, TFRAG], BF16)
              for q in range(2)]
    a2a_out = [nc.dram_tensor(f"a2a_out{q}", [NCORES * 128, TFRAG], BF16)
               for q in range(2)]
    a2a_in_q = [nc.dram_tensor(f"a2a_inq{q}", [NCORES * 128, TFRAG // 2],
                               BF16) for q in range(4)]
    a2a_out_q = [nc.dram_tensor(f"a2a_outq{q}", [NCORES * 128, TFRAG // 2],
                                BF16) for q in range(4)]
    warm_in = nc.dram_tensor("warm_in", [NCORES, 4], BF16)
    warm_out = nc.dram_tensor("warm_out", [NCORES, 4], BF16)
    rcp_d = [nc.dram_tensor(f"rcp_d{s}", [1, ITILE], BF16) for s in range(4)]

    if DEBUG_DUMP:
        dbg_qk = nc.dram_tensor("dbg_qk", [128, 2 * NSEQ], BF16,
                                kind="ExternalOutput")
        dbg_vn = nc.dram_tensor("dbg_vn", [128, NJ * 130], BF16,
                                kind="ExternalOutput")
        dbg_ouc = nc.dram_tensor("dbg_ouc", [65, 8 * ITILE], F32,
                                 kind="ExternalOutput")
        dbg_outT = nc.dram_tensor("dbg_outT", [128, T], BF16,
                                  kind="ExternalOutput")

    with tile.TileContext(nc) as tc, ExitStack() as ctx:
        ep = ctx.enter_context

        consts = ep(tc.tile_pool(name="consts", bufs=1))
        p_exp = ep(tc.tile_pool(name="exps", bufs=4))
        p_ouc = ep(tc.tile_pool(name="ouc", bufs=6))
        p_small = ep(tc.tile_pool(name="small", bufs=4))
        p_ots = ep(tc.tile_pool(name="ots", bufs=4))
        p_y = ep(tc.tile_pool(name="ysb", bufs=2))
        ps_s = ep(tc.tile_pool(name="pss", bufs=2, space="PSUM"))
        ps_ou = ep(tc.tile_pool(name="psou", bufs=2, space="PSUM"))
        ps_mm = ep(tc.tile_pool(name="psmm", bufs=2, space="PSUM"))

        # ---- weights / constants to SBUF ----
        wqk_sb = consts.tile([128, NKC * 256], BF16, name="wqk_sb")
        wv_sb = consts.tile([128, NKC * 128], BF16, name="wv_sb")
        wp_sb = consts.tile([128, NKC * C], BF16, name="wp_sb")
        bp_sb = consts.tile([1, C], BF16, name="bp_sb")

        # x^T: one tile per batch, layout [:, c*2048 + t]
        xt = [consts.tile([128, NKC * NSEQ], BF16, name=f"xt{b}")
              for b in range(B)]

        def load_xu(b, u, cs, eng):
            """Plain 2D DMA of one u-slice (512 tokens) of x^T chunks."""
            for c in cs:
                eng.dma_start(
                    out=xt[b][:, c * NSEQ + u * ITILE:
                              c * NSEQ + (u + 1) * ITILE],
                    in_=xT_d[c * 128:(c + 1) * 128,
                             b * NSEQ + u * ITILE: b * NSEQ + (u + 1) * ITILE])

        def load_wqk(c0, c1, eng):
            eng.dma_start(
                out=wqk_sb[:].rearrange("p (c n) -> p c n",
                                        c=NKC)[:, c0:c1],
                in_=wqk_d[c0 * 128:c1 * 128, :].rearrange(
                    "(c p) n -> p c n", p=128))

        # Startup DMA plan: u-major x slices so each qkv unit's 8 c-chunks
        # arrive together, spread across the three DMA-capable queues
        # (sync/SP, scalar, gpsimd) in first-use order.
        load_wqk(0, 4, nc.sync)
        load_wqk(4, 8, nc.gpsimd)
        # Dummy collective: absorbs the ~11.5us first-collective warmup
        # delay during the qkv phase so a2a(0) starts promptly.
        nc.gpsimd.collective_compute(
            "AllToAll", mybir.AluOpType.bypass,
            replica_groups=[list(range(NCORES))],
            ins=[warm_in[:, :]], outs=[warm_out[:, :]])
        nc.gpsimd.dma_start(
            out=wv_sb[:].rearrange("p (c n) -> p c n", c=NKC),
            in_=wv_d[:, :].rearrange("(c p) n -> p c n", p=128))
        for u in range(4):
            load_xu(0, u, range(0, 4), nc.sync)
            load_xu(0, u, range(4, 8), nc.scalar)
        nc.sync.dma_start(out=bp_sb[:], in_=bp_d[0:1, :])
        for u in range(4):
            load_xu(1, u, range(0, 8), nc.gpsimd)
        nc.sync.dma_start(
            out=wp_sb[:].rearrange("p (c n) -> p c n", c=NKC),
            in_=wp_d[:, :].rearrange("(c p) n -> p c n", p=128))

        ident = consts.tile([128, 128], BF16, name="ident")
        make_identity(nc, ident[:])
        onesc = consts.tile([1, 128], BF16, name="onesc")
        nc.vector.memset(onesc[:], 1.0)

        # bias broadcast [128, C] f32, computed once via 1x128 outer product
        bias_bc = consts.tile([128, C], F32, name="bias_bc")

        def make_bias_bc():
            for n in range(2):
                bps = ps_mm.tile([128, ITILE], F32, tag="mm", name="bps")
                nc.tensor.matmul(bps[:], onesc[:],
                                 bp_sb[:, n * ITILE:(n + 1) * ITILE],
                                 start=True, stop=True)
                nc.vector.tensor_copy(bias_bc[:, n * ITILE:(n + 1) * ITILE],
                                      bps[:])

        # ---- persistent per-batch / per-chunk state ----
        qz = [[consts.tile([128, NSEQ], BF16, name=f"qz{b}{h}")
               for h in range(2)] for b in range(B)]
        kz = [[consts.tile([128, NSEQ], BF16, name=f"kz{b}{h}")
               for h in range(2)] for b in range(B)]
        for b in range(B):
            nc.vector.memset(qz[b][0][64:128, :], 0.0)
            nc.vector.memset(kz[b][0][64:128, :], 0.0)
            nc.vector.memset(qz[b][1][0:64, :], 0.0)
            nc.vector.memset(kz[b][1][0:64, :], 0.0)
        vT = [consts.tile([128, NSEQ], BF16, name=f"vT{b}") for b in range(B)]
        # vn[j]: [v_A(64) | 1 | v_B(64) | 1 | zeros(63)]; constants written once
        vn = [consts.tile([128, 193], BF16, name=f"vn{j}") for j in range(NJ)]
        for j in range(NJ):
            nc.vector.memset(vn[j][:, 64:65], 1.0)
            nc.vector.memset(vn[j][:, 129:130], 1.0)
            nc.vector.memset(vn[j][:, 130:193], 0.0)
        # normalized attention output, per head (partitions 0-63)
        outT = [consts.tile([64, T], BF16, name=f"outT{h}") for h in range(2)]

        def xts(b, u, c):
            return xt[b][:, c * NSEQ + u * ITILE: c * NSEQ + (u + 1) * ITILE]

        # ---- qkv projection: one (w, u) unit = 8 matmuls + 1 evac ----
        def qkv_unit(b, tp, w, uu):
            def emit():
                u = 2 * tp + uu
                usl = slice(u * ITILE, (u + 1) * ITILE)
                pst = ps_mm.tile([128, ITILE], F32, tag="mm", name="pst")
                for c in range(NKC):
                    if w < 2:
                        lhsT = wqk_sb[:, c * 256 + w * 128:
                                      c * 256 + (w + 1) * 128]
                    else:
                        lhsT = wv_sb[:, c * 128:(c + 1) * 128]
                    nc.tensor.matmul(pst[:], lhsT, xts(b, u, c),
                                     start=(c == 0), stop=(c == NKC - 1))
                if w == 2:
                    nc.vector.tensor_copy(vT[b][:, usl], pst[:])
                else:
                    dst = (qz, kz)[w][b]
                    nc.vector.tensor_copy(dst[0][0:64, usl], pst[0:64, :])
                    nc.vector.tensor_copy(dst[1][64:128, usl],
                                          pst[64:128, :])
            return emit

        def qkv_units(b, tp, ws=(0, 1, 2), uu_major=False):
            if uu_major:
                return [qkv_unit(b, tp, w, uu) for uu in range(2) for w in ws]
            return [qkv_unit(b, tp, w, uu) for w in ws for uu in range(2)]

        # ---- vn construction: one unit = 2 transposes + 4 copies ----
        # (PE transposes: DMA-xbar transposes mid-schedule corrupt results,
        # Tile's transpose/collective serialization cannot handle them)
        def vn_unit(b, tcn0):
            def emit():
                for tcn in (tcn0, tcn0 + 1):
                    vtr = ps_mm.tile([128, 128], BF16, tag="mm", name="vtr")
                    nc.tensor.transpose(vtr[:],
                                        vT[b][:, tcn * 128:(tcn + 1) * 128],
                                        ident[:])
                    nc.vector.tensor_copy(vn[tcn][:, 0:64], vtr[:, 0:64])
                    nc.vector.tensor_copy(vn[tcn][:, 65:129], vtr[:, 64:128])
            return emit

        def vn_units(b, tcns):
            return [vn_unit(b, t0) for t0 in tcns]

        # ---- attention ----
        outUc = {}

        def attn_pairs(b, i, sub=None):
            """Generator: one yield per j-chunk pair (8 per i-tile).
            sub=0/1 processes only a 256-query half of the i-tile (used to
            shrink the final exchanged fragment)."""
            if sub is None:
                q0, W = i * ITILE, ITILE
            else:
                q0, W = i * ITILE + sub * (ITILE // 2), ITILE // 2
            isl = slice(q0, q0 + W)
            outu = [ps_ou.tile([128, W], F32, tag="ou", name="outu")
                    for _ in range(2)]
            for g in range(NJ // 2):
                sts = []
                for jj in (2 * g, 2 * g + 1):
                    s_t = ps_s.tile([128, 2 * W], F32, tag="s", name="s_t")
                    for h in range(2):
                        nc.tensor.matmul(
                            s_t[:, h * W:(h + 1) * W],
                            kz[b][h][:, jj * 128:(jj + 1) * 128],
                            qz[b][h][:, isl],
                            start=True, stop=True)
                    sts.append(s_t)
                exs = []
                for k in range(2):
                    ex = p_exp.tile([128, 2 * W], BF16, tag="ex", name="ex")
                    nc.scalar.activation(ex[:], sts[k][:], EXP, scale=SCALE)
                    exs.append(ex)
                for k, jj in enumerate((2 * g, 2 * g + 1)):
                    for h in range(2):
                        nc.tensor.matmul(
                            outu[h][:],
                            vn[jj][:, h * 65: h * 65 + 128],
                            exs[k][:, h * W:(h + 1) * W],
                            start=(jj == 0), stop=(jj == NJ - 1))
                yield
            last = (b, i) == (B - 1, NI - 1) and sub in (None, 1)
            for h in range(2):
                slot = (b * 8 + i * 2 + h) % 4
                if last and not DEBUG_DUMP:
                    # last sub-tile: run the whole reciprocal chain inline
                    # (tail latency) and read PSUM directly -- nothing
                    # reuses this PSUM afterwards, no evac copy needed.
                    lnt = p_small.tile([65, W], F32, tag="rcp", name="lnt")
                    nc.scalar.activation(lnt[64:65, :], outu[h][64:65, :],
                                         LN)
                    rcpb = p_small.tile([65, W], BF16, tag="rcpb",
                                        name="rcpb")
                    nc.scalar.activation(rcpb[64:65, :], lnt[64:65, :],
                                         EXP, scale=-1.0)
                    nc.sync.dma_start(out=rcp_d[slot][0:1, 0:W],
                                      in_=rcpb[64:65, :])
                    outUc[(b, i, h, sub)] = (outu[h], slot, q0, W)
                    continue
                ouc = p_ouc.tile([65, W], F32, tag="ouc", name="ouc")
                nc.vector.tensor_copy(ouc[:], outu[h][0:65, :])
                outUc[(b, i, h, sub)] = (ouc, slot, q0, W)
                if DEBUG_DUMP and b == 0:
                    sl = (i * 2 + h) * ITILE
                    nc.sync.dma_start(out=dbg_ouc[:, sl:sl + W],
                                      in_=ouc[:])

        def norm_rcp(b, i, h, sub=None):
            """1/den = exp(-ln(den)) on ScalarE from the ouc copy.  Emitted
            as a filler in the NEXT drive so the two extra ScalarE ops don't
            wedge between two i-tiles' exp streams and stall the AV refill
            (Ln and Exp share one table set -- no table switching)."""
            def emit():
                ouc, slot, q0, W = outUc[(b, i, h, sub)]
                lnt = p_small.tile([65, W], F32, tag="rcp", name="lnt")
                nc.scalar.activation(lnt[64:65, :], ouc[64:65, :], LN)
                rcpb = p_small.tile([65, W], BF16, tag="rcpb", name="rcpb")
                nc.scalar.activation(rcpb[64:65, :], lnt[64:65, :], EXP,
                                     scale=-1.0)
                nc.sync.dma_start(out=rcp_d[slot][0:1, 0:W],
                                  in_=rcpb[64:65, :])
            return emit

        def norm_mul(b, i, h, sub=None):
            """DMA broadcast of 1/den + DVE multiply; scheduled a few slots
            after the i-tile so the rcp_d write latency is hidden."""
            def emit():
                t0 = b * NSEQ
                ouc, slot, q0, W = outUc.pop((b, i, h, sub))
                bc_sb = p_small.tile([64, W], BF16, tag="bc", name="bcsb")
                nc.sync.dma_start(
                    out=bc_sb[:],
                    in_=rcp_d[slot][0:1, 0:W].to_broadcast((64, W)))
                nc.vector.tensor_mul(
                    outT[h][0:64, t0 + q0: t0 + q0 + W],
                    ouc[0:64, :], bc_sb[:])
            return emit

        # All CC-gated DMAs (ots loads, y stores) live on the gpsimd DGE
        # queue in natural order (trigger q -> ots q -> trigger q+1 ...):
        # their semaphore waits are harmless there because everything queued
        # behind them is gated even later.  On sync they head-of-line-block
        # the norm broadcasts; on scalar they stall the exp pipeline.
        otsT = {}

        def stage_a2a(q):
            """Half-batch collective for batch 0 (tokens q*1024..)."""
            hs = q * HALF
            for h in range(2):
                nc.sync.dma_start(
                    out=a2a_in[q][:, :].rearrange(
                        "(s x) t -> x s t", s=NCORES)[h * 64:(h + 1) * 64],
                    in_=outT[h][:, hs: hs + HALF].rearrange(
                        "p (s t) -> p s t", s=NCORES))
            nc.gpsimd.collective_compute(
                "AllToAll", mybir.AluOpType.bypass,
                replica_groups=[list(range(NCORES))],
                ins=[a2a_in[q][:, :]], outs=[a2a_out[q][:, :]])
            ots = p_ots.tile([128, NCORES * TFRAG], BF16, tag="ots",
                             name="ots")
            nc.gpsimd.dma_start(
                out=ots[:].rearrange("p (s t) -> p s t", s=NCORES),
                in_=a2a_out[q][:, :].rearrange("(s p) t -> p s t",
                                               s=NCORES))
            otsT[q] = ots

        def stage_a2a_q(qq):
            """i-tile-sized collective for batch 1 (tokens 2048+qq*512..)."""
            hs = NSEQ + qq * ITILE
            for h in range(2):
                nc.sync.dma_start(
                    out=a2a_in_q[qq][:, :].rearrange(
                        "(s x) t -> x s t", s=NCORES)[h * 64:(h + 1) * 64],
                    in_=outT[h][:, hs: hs + ITILE].rearrange(
                        "p (s t) -> p s t", s=NCORES))
            nc.gpsimd.collective_compute(
                "AllToAll", mybir.AluOpType.bypass,
                replica_groups=[list(range(NCORES))],
                ins=[a2a_in_q[qq][:, :]], outs=[a2a_out_q[qq][:, :]])

        def load_ots_q2(qq0):
            """Pair two 64-token fragments (qq0, qq0+1) into one ots tile."""
            ots = p_ots.tile([128, NCORES * TFRAG], BF16, tag="ots",
                             name="otsq")
            tf = TFRAG // 2
            for k in range(2):
                nc.gpsimd.dma_start(
                    out=ots[:].rearrange("p (s t) -> p s t",
                                         s=NCORES)[:, :, k * tf:(k + 1) * tf],
                    in_=a2a_out_q[qq0 + k][:, :].rearrange(
                        "(s p) t -> p s t", s=NCORES))
            otsT[2 + qq0 // 2] = ots

        def oproj(q):
            # alternate the PSUM pool (ps_mm / ps_ou, both idle in the
            # tail) so consecutive oproj units don't convoy on the 2-buffer
            # ring waiting for the previous unit's bias-adds
            def emit():
                ots = otsT.pop(q)
                pool, tg = [(ps_mm, "mm"), (ps_ou, "ou"),
                            (ps_s, "s"), (ps_ou, "ou")][q]
                y_ps = [pool.tile([128, ITILE], F32, tag=tg, name="yps")
                        for _ in range(2)]
                y_sb = p_y.tile([128, C], F32, tag="y", name="ysb")
                yr0 = q * TFRAG
                # n-outer so the first half's bias-add + y store overlap
                # the second half's matmul chain (shrinks the exposed tail)
                for n in range(2):
                    for s in range(NKC):
                        nc.tensor.matmul(
                            y_ps[n][:],
                            ots[:, s * TFRAG:(s + 1) * TFRAG],
                            wp_sb[:, s * C + n * ITILE: s * C + (n + 1) * ITILE],
                            start=(s == 0), stop=(s == NKC - 1))
                    nc.vector.tensor_add(y_sb[:, n * ITILE:(n + 1) * ITILE],
                                         y_ps[n][:],
                                         bias_bc[:, n * ITILE:(n + 1) * ITILE])
                    # y stores go on sync: on gpsimd they sit behind
                    # CC-gated ots loads and keep y_sb ring buffers alive
                    nc.sync.dma_start(
                        out=y_d[yr0: yr0 + TFRAG, n * ITILE:(n + 1) * ITILE],
                        in_=y_sb[:, n * ITILE:(n + 1) * ITILE])
            return emit

        def drive(gen, fillers):
            """Interleave: one filler unit emitted BEFORE each attention
            j-pair; leftovers drain after the i-tile. None = empty slot."""
            fl = deque(fillers)
            while True:
                if fl:
                    f = fl.popleft()
                    if f is not None:
                        f()
                try:
                    next(gen)
                except StopIteration:
                    break
            while fl:
                f = fl.popleft()
                if f is not None:
                    f()

        def seq(*fns):
            def emit():
                for f in fns:
                    f()
            return emit

        def a2a_unit(q):
            def emit():
                stage_a2a(q)
            return emit

        # ---- emission schedule ----
        for f in qkv_units(0, 0, uu_major=True):
            f()
        for f in vn_units(0, range(0, 8, 2)):
            f()

        drive(attn_pairs(0, 0),
              qkv_units(0, 1, ws=(2, 1)) + vn_units(0, range(8, 16, 2))
              + qkv_units(0, 1, ws=(0,)))
        if DEBUG_DUMP:
            nc.sync.dma_start(out=dbg_qk[0:64, 0:NSEQ], in_=qz[0][0][0:64, :])
            nc.sync.dma_start(out=dbg_qk[64:128, 0:NSEQ],
                              in_=qz[0][1][64:128, :])
            nc.sync.dma_start(out=dbg_qk[0:64, NSEQ:], in_=kz[0][0][0:64, :])
            nc.sync.dma_start(out=dbg_qk[64:128, NSEQ:],
                              in_=kz[0][1][64:128, :])
            for j in range(NJ):
                nc.sync.dma_start(out=dbg_vn[:, j * 130:j * 130 + 130],
                                  in_=vn[j][:, 0:130])

        drive(attn_pairs(0, 1),
              [make_bias_bc, seq(norm_rcp(0, 0, 0), norm_rcp(0, 0, 1)),
               None, norm_mul(0, 0, 0), norm_mul(0, 0, 1)])
        q10 = qkv_units(1, 0)
        drive(attn_pairs(0, 2),
              [q10[0], seq(norm_rcp(0, 1, 0), norm_rcp(0, 1, 1)), q10[1],
               seq(norm_mul(0, 1, 0), norm_mul(0, 1, 1), a2a_unit(0)),
               q10[2], q10[3], q10[4], q10[5]])
        # vn_units(1, (0,1)) must sit at slot >= 1: the slot-k filler is
        # emitted BEFORE attention pair k, and pair 0 still reads batch-0
        # vn[0..1] -- emitting the overwrite first would reorder the data.
        drive(attn_pairs(0, 3),
              [seq(norm_rcp(0, 2, 0), norm_rcp(0, 2, 1))]
              + vn_units(1, range(0, 4, 2))
              + [seq(norm_mul(0, 2, 0), norm_mul(0, 2, 1))]
              + vn_units(1, range(4, 8, 2)))
        qk11 = qkv_units(1, 1, ws=(2, 1))
        vn1hi = vn_units(1, range(8, 16, 2))
        q11q = qkv_units(1, 1, ws=(0,))
        drive(attn_pairs(1, 0),
              [seq(norm_rcp(0, 3, 0), norm_rcp(0, 3, 1)),
               qk11[0], qk11[1],
               seq(norm_mul(0, 3, 0), norm_mul(0, 3, 1), a2a_unit(1)),
               seq(qk11[2], vn1hi[0]), seq(qk11[3], vn1hi[1]),
               vn1hi[2], vn1hi[3], q11q[0], q11q[1]])
        drive(attn_pairs(1, 1),
              [seq(norm_rcp(1, 0, 0), norm_rcp(1, 0, 1)), None,
               seq(norm_mul(1, 0, 0), norm_mul(1, 0, 1)),
               lambda: stage_a2a_q(0)])
        drive(attn_pairs(1, 2),
              [seq(norm_rcp(1, 1, 0), norm_rcp(1, 1, 1)), None,
               seq(norm_mul(1, 1, 0), norm_mul(1, 1, 1)),
               lambda: (stage_a2a_q(1), load_ots_q2(0))])
        drive(attn_pairs(1, 3),
              [seq(norm_rcp(1, 2, 0), norm_rcp(1, 2, 1)), None,
               seq(norm_mul(1, 2, 0), norm_mul(1, 2, 1)),
               lambda: stage_a2a_q(2)])
        norm_mul(1, 3, 0)(); norm_mul(1, 3, 1)()
        if DEBUG_DUMP:
            nc.sync.dma_start(out=dbg_outT[0:64, :], in_=outT[0][:])
            nc.sync.dma_start(out=dbg_outT[64:128, :], in_=outT[1][:])
        stage_a2a_q(3)
        # deferred output projections: 0-2 fill the PE while the final
        # collective is in flight; 3 runs as soon as its data lands.
        # tile_wait_until pins them to the END of the static schedule --
        # otherwise the scheduler interleaves their matmuls into the
        # attention stream (their a2a data is long ready), delaying the
        # last ACTIVATE that gates the final collective by ~13us, and
        # reorders the ots loads ahead of the final collective trigger on
        # the gpsimd queue.  Runtime order is still semaphore-driven.
        with tc.tile_wait_until(1.0):
            load_ots_q2(2)
            oproj(0)()
            oproj(1)()
            oproj(2)()
            oproj(3)()

    nc.compile()
    return nc


_NC = None


def _get_nc():
    global _NC
    if _NC is None:
        _NC = build_program()
    return _NC


def prep_in_maps(x, w_qkv, w_proj, b_proj):
    x = np.asarray(x, dtype=np.float32).reshape(T, C)
    xT_bf = np.ascontiguousarray(x.T).astype(NPBF16)          # [C, T]
    w_qkv = np.asarray(w_qkv, dtype=np.float32)
    w_proj = np.asarray(w_proj, dtype=np.float32)
    b_proj = np.asarray(b_proj, dtype=np.float32)
    wp_bf = np.ascontiguousarray(w_proj).astype(NPBF16)
    bp_bf = b_proj.reshape(1, C).astype(NPBF16)

    q_w, k_w, v_w = w_qkv[:, 0:C], w_qkv[:, C:2 * C], w_qkv[:, 2 * C:3 * C]
    in_maps = []
    for c in range(NCORES):
        hA, hB = 2 * c, 2 * c + 1
        sA, sB = slice(hA * D, (hA + 1) * D), slice(hB * D, (hB + 1) * D)
        wqk_c = np.concatenate([q_w[:, sA], q_w[:, sB], k_w[:, sA], k_w[:, sB]],
                               axis=1).astype(NPBF16)
        wv_c = np.concatenate([v_w[:, sA], v_w[:, sB]], axis=1).astype(NPBF16)
        in_maps.append({"x": xT_bf, "wqk": np.ascontiguousarray(wqk_c),
                        "wv": np.ascontiguousarray(wv_c), "wproj": wp_bf,
                        "bproj": bp_bf})
    return in_maps


def assemble(results):
    y = np.empty((T, C), dtype=np.float32)
    tf = TFRAG // 2
    for c in range(NCORES):
        yc = results[c]["y"]
        # batch 0: two half-batch fragments of 128 tokens
        for q in range(2):
            g0 = q * HALF + c * TFRAG
            y[g0: g0 + TFRAG, :] = yc[q * TFRAG: (q + 1) * TFRAG, :]
        # batch 1: four i-tile fragments of 64 tokens
        for qq in range(4):
            g0 = NSEQ + qq * ITILE + c * tf
            r0 = 2 * TFRAG + qq * tf
            y[g0: g0 + tf, :] = yc[r0: r0 + tf, :]
    return y.reshape(B, NSEQ, C)


def run(in_maps, trace=False):
    nc = _get_nc()
    return run_bass_kernel_spmd(nc, in_maps, core_ids=list(range(NCORES)),
                                trace=trace)


def kernel(x, w_qkv, w_proj, b_proj):
    res = run(prep_in_maps(x, w_qkv, w_proj, b_proj))
    return assemble(res.results)
